# revision 19
# baseline (speedup 1.0000x reference)
"""Multi-head self-attention (b=2, n=2048, emb=1024, heads=16) on 8 trn2 cores.

Sharding: core c = (b, hg) with b = c // 4, hg = c % 4. Data parallel over
batch, tensor parallel over head-groups (4 heads / 256 emb-cols per core).
Each core computes Q/K/V projections for its heads, full attention for its
heads, and a partial output projection ctx_hg @ Wo[:, hg_slice].T of shape
[2048, 1024]. The host sums the 4 partials per batch (Megatron row-parallel
reduce done on host) and adds the rank-1 bias term bv @ Wo.T + bo.

Key device-side structure (v2):
- x^T resident in SBUF; fp16 matmuls everywhere; q/k biases fused into the
  PSUM->SBUF copies; v/o biases are the host-side rank-1 term.
- S matmuls are ROW-PACKED: the two heads of a pair live in partition rows
  0:64 / 64:128 of qT/kT, and their S matmuls are emitted head-interleaved
  so adjacent matmuls target disjoint 64-row groups of the PE array
  (tile_position auto-derived from base_partition) and stream concurrently
  -> a packed pair costs ~512 cycles instead of 2x512.
- The two heads' exps run on DIFFERENT engines concurrently: one on ACT
  (true exp, 1024-wide out of PSUM) and one on DVE via a custom
  Schraudolph op: e = bitcast_fp16(int16(relu(S*(scale*1024/ln2)+15315.3)))
  (max rel err ~3%, mean ~1.8%, HW-validated). Which head uses which
  engine alternates per work item, so the approx noise averages across
  nk-chunks inside every head instead of concentrating in one head
  (absmax-rel error ~2.4e-3 in numpy sim). This halves the exp-chain
  period AND lets both S psum tags recycle each item.
- V carries a ones column per head -> ctx matmul row 64 accumulates the
  softmax denominators for free; normalize = rowsum -> partition 0 (DVE),
  gpsimd broadcast, reciprocal_approx_fast (DVE), multiply (gpsimd from the
  staged SBUF copy; last window straight from PSUM on DVE to shorten the
  tail chain).
- Engine balance: out-proj PSUM->SBUF copies run on ACT (scalar.copy),
  normalize multiplies on gpsimd, kq-bias adds + v copies + exp(DVE half)
  + reciprocal on DVE; out_part stores issue on the sync DMA queue.
- Projection/out-proj parcels fill PE slack inside the attention windows
  from per-window filler lists (emission-deadline ordered, paced ~2/item).
"""

import os
import sys

for _p in ("/opt/trn_rl_repo", "/root/.axon_site/_ro/trn_rl_repo"):
    if os.path.isdir(_p) and _p not in sys.path:
        sys.path.append(_p)

import numpy as np

import concourse.bass as bass  # noqa: F401  (engine types pulled via nc)
import concourse.mybir as mybir
import concourse.tile as tile
from concourse import bacc
from concourse.bass_utils import run_bass_kernel_spmd
from concourse import dve_ops
from concourse.dve_spec import Spec, Src0, C0, C1, relu, lower
from concourse.dve_uop import DveOpSpec

B, N, EMB, HEADS, HD = 2, 2048, 1024, 16, 64
N_CORES = 8
TP = 4                      # head-group shards per batch
DQ = EMB // TP              # 256 emb-cols (4 heads) per core
SCALE = HD ** -0.5          # 0.125

F32 = mybir.dt.float32
F16 = mybir.dt.float16
I16 = mybir.dt.int16
FP = mybir.ActivationFunctionType

NQ = 512                    # nq chunk for projections / out-proj
NJ = N // NQ                # 4 nq chunks
NQA = 512                   # nq chunk for attention
NJA = N // NQA              # 4 attention nq chunks
NKC = 128                   # nk chunk (ctx contraction)
NT = N // NKC               # 16 nk chunks
KC = EMB // 128             # 8 e chunks
GK = 2                      # nk chunks per S-psum group (1024-wide exps)
NG = NT // GK               # 8 groups per (head-pair, j)

# Schraudolph fp16-bit exp: bits = relu(x*SCALE*1024/ln2 + (15360-44.7))
A_EXP = float(SCALE * 1024.0 / np.log(2.0))
B_EXP = 15360.0 - 44.7


def _register_exp_op():
    for op in dve_ops.OPS:
        if op.name == "EXP_SCHRAUDOLPH_ANT":
            return op

    def _ref(in0, in1, c0, c1, c2):
        return np.maximum(in0 * c0 + c1, 0.0)

    spec = Spec(body=relu(Src0 * C0 + C1), reference=_ref)
    shas = {}
    for ver in ("v3", "v4"):
        tmp = DveOpSpec(name="EXP_SCHRAUDOLPH_ANT", opcode=1,
                        uops=lower(spec, ver=ver), rd1_en=False)
        shas[ver] = tmp.sha(ver)
    op = dve_ops.DveOp("EXP_SCHRAUDOLPH_ANT", spec, subdim=False, uops_sha=shas)
    dve_ops.OPS.append(op)
    dve_ops.CUSTOM_DVE_SPECS[op.name] = op.spec
    dve_ops._SUB_OPCODE_FOR_NAME[op.name] = (
        dve_ops._CUSTOM_DVE_ROW_BASE + len(dve_ops.OPS) - 1)
    return op


EXP_OP = _register_exp_op()


def build_program():
    """Build + compile the single SPMD program all 8 cores run."""
    nc = bacc.Bacc("TRN2", target_bir_lowering=False, debug=False,
                   num_devices=N_CORES)

    xT = nc.dram_tensor("xT", [EMB, N], F16, kind="ExternalInput").ap()
    wqT = nc.dram_tensor("wqT", [EMB, DQ], F16, kind="ExternalInput").ap()
    wkT = nc.dram_tensor("wkT", [EMB, DQ], F16, kind="ExternalInput").ap()
    wvT = nc.dram_tensor("wvT", [EMB, DQ], F16, kind="ExternalInput").ap()
    woT = nc.dram_tensor("woT", [DQ, EMB], F16, kind="ExternalInput").ap()
    bqd = nc.dram_tensor("bq_s", [DQ], F32, kind="ExternalInput").ap()
    bkd = nc.dram_tensor("bk_s", [DQ], F32, kind="ExternalInput").ap()
    # fp16 partials: host sums 4 of them in fp32; quantization of the
    # partial (|.| ~ 1, ulp ~ 1e-3) adds ~1e-3 absmax-relative error --
    # well under the 2e-2 gate -- and halves the store traffic + tail.
    out_part = nc.dram_tensor("out_part", [N, EMB], F16,
                              kind="ExternalOutput").ap()

    with tile.TileContext(nc) as tc:
        with (
            tc.tile_pool(name="const", bufs=1) as const,
            tc.tile_pool(name="persist", bufs=1) as persist,
            tc.tile_pool(name="epool", bufs=3) as epool,
            tc.tile_pool(name="npool", bufs=2) as npool,
            tc.tile_pool(name="opool", bufs=4) as opool,
            # PSUM static budget (8 banks): pp 2 + s0 2 + s1 2 + c0 1 + c1 1
            tc.tile_pool(name="ppool", bufs=2, space="PSUM") as ppool,
            tc.tile_pool(name="spool", bufs=1, space="PSUM") as spool,
            tc.tile_pool(name="cpool", bufs=1, space="PSUM") as cpool,
        ):
            # ---- constants + resident x ----
            # first projection group needs wk chunks + x n-slice 0 only; those
            # DMAs go first, fine-grained, spread across the Sync / GpSimd /
            # Scalar queues so no single sequencer's DGE time serializes the
            # startup.
            wq_sb = const.tile([128, KC, DQ], F16, tag="wq")
            wk_sb = const.tile([128, KC, DQ], F16, tag="wk")
            wv_sb = const.tile([128, KC, DQ], F16, tag="wv")
            x_sb = const.tile([128, KC, N], F16, tag="x")
            xTr = xT.rearrange("(k p) n -> k p n", p=128)
            bk_sb = const.tile([128, 2], F32, tag="bk")
            bq_sb = const.tile([128, 2], F32, tag="bq")
            # Wave plan (3 DGE queues: sync/scalar/gpsimd, ~0.6us/desc):
            # wave 1 carries the first kq_group's needs (wk + x0) plus wq
            # (needed right after on PE) in per-chunk arrival order; wv and
            # the later x slices follow before their first consumers.
            nc.scalar.dma_start(out=bk_sb, in_=bkd.rearrange("(m p) -> p m", p=128))
            nc.scalar.dma_start(out=bq_sb, in_=bqd.rearrange("(m p) -> p m", p=128))
            wkr = wkT.rearrange("(k p) d -> k p d", p=128)
            wqr = wqT.rearrange("(k p) d -> k p d", p=128)
            wvr = wvT.rearrange("(k p) d -> k p d", p=128)
            for k in range(KC):
                nc.sync.dma_start(out=wk_sb[:, k, :], in_=wkr[k])
                nc.gpsimd.dma_start(out=x_sb[:, k, 0:NQ], in_=xTr[k, :, 0:NQ])
                nc.scalar.dma_start(out=wq_sb[:, k, :], in_=wqr[k])
            for k in range(KC):
                nc.sync.dma_start(out=x_sb[:, k, NQ:2 * NQ],
                                  in_=xTr[k, :, NQ:2 * NQ])
                nc.gpsimd.dma_start(out=wv_sb[:, k, :], in_=wvr[k])
                nc.scalar.dma_start(out=x_sb[:, k, 2 * NQ:3 * NQ],
                                    in_=xTr[k, :, 2 * NQ:3 * NQ])
            for k in range(KC):
                nc.gpsimd.dma_start(out=x_sb[:, k, 3 * NQ:N],
                                    in_=xTr[k, :, 3 * NQ:N])
            # wo is needed only by the out-projection (~100us in) -- deferred
            # into the filler stream to keep startup queues clear
            wo_sb = const.tile([128, 2, EMB], F16, tag="wo")

            # ---- persistent activations ----
            qT = [persist.tile([128, N], F16, tag=f"qT{p}", name=f"qT{p}") for p in range(2)]
            kT = [persist.tile([128, N], F16, tag=f"kT{p}", name=f"kT{p}") for p in range(2)]
            ctxT = [persist.tile([128, N], F16, tag=f"ctxT{p}", name=f"ctxT{p}") for p in range(2)]
            # V for all 4 local heads: [nk-part, t, head*65 + (0:64 | ones)]
            v_all = persist.tile([128, NT, 4 * (HD + 1)], F16, tag="v")
            for h in range(4):
                nc.vector.memset(v_all[:, :, h * 65 + 64], 1.0)

            add, mult = mybir.AluOpType.add, mybir.AluOpType.mult

            # ---- projection building blocks ----
            # Each is one PSUM accumulation group on the double-buffered pp
            # tag, small enough to slot between attention groups.
            def kq_group(p, n, wsb, bsb, dst):
                ps = ppool.tile([128, NQ], F32, tag="pp", name="kqp")
                for k in range(KC):
                    nc.tensor.matmul(
                        ps, wsb[:, k, p * 128:(p + 1) * 128],
                        x_sb[:, k, n * NQ:(n + 1) * NQ],
                        start=(k == 0), stop=(k == KC - 1))
                nc.vector.tensor_tensor(
                    out=dst[p][:, n * NQ:(n + 1) * NQ], in0=ps,
                    in1=bsb[:, p:p + 1].broadcast_to([128, NQ]), op=add)

            def v_group(n, tl):
                # V for ALL 4 local heads at once (256-col moving wv)
                t = n * 4 + tl
                ps = ppool.tile([128, DQ], F32, tag="pp", name="vp")
                for k in range(KC):
                    nc.tensor.matmul(
                        ps, x_sb[:, k, t * 128:(t + 1) * 128],
                        wv_sb[:, k, :],
                        start=(k == 0), stop=(k == KC - 1))
                vv = v_all[:, t, :].rearrange("p (h c) -> p h c", c=65)
                nc.vector.tensor_copy(
                    out=vv[:, :, 0:64],
                    in_=ps.rearrange("p (h c) -> p h c", c=64))

            # Minimal cold prefix: only the work attention j=0 strictly
            # needs before its first items runs up front -- K(n=0), Q(n=0),
            # V(t=0,1). Everything else streams through the hot attention
            # windows as fillers.
            kq_group(0, 0, wk_sb, bk_sb, kT)
            kq_group(0, 0, wq_sb, bq_sb, qT)
            v_group(0, 0)
            v_group(0, 1)

            # ---- out-projection ----
            # one (m, eo) parcel: both head-pair passes accumulate into a
            # single pp-tag PSUM group, then one ACT copy to SBUF + store.
            def out_proj_parcel(m, eo):
                po = ppool.tile([128, NQ], F32, tag="pp", name="po")
                for kp in range(2):
                    nc.tensor.matmul(
                        po, ctxT[kp][:, m * 128:(m + 1) * 128],
                        wo_sb[:, kp, eo * NQ:(eo + 1) * NQ],
                        start=(kp == 0), stop=(kp == 1))
                o = opool.tile([128, NQ], F16, tag="o", name="o")
                nc.scalar.copy(out=o, in_=po)
                nc.sync.dma_start(
                    out=out_part[m * 128:(m + 1) * 128, eo * NQ:(eo + 1) * NQ],
                    in_=o)

            # The LAST window's m-chunks split the parcel: the ctxT0 half
            # runs during the window (ctxT0 is final since p0), leaving only
            # one matmul + add + store per parcel on the serial tail.
            o0s = {}

            def oproj_kp0(m, eo):
                po = ppool.tile([128, NQ], F32, tag="pp", name="po")
                nc.tensor.matmul(
                    po, ctxT[0][:, m * 128:(m + 1) * 128],
                    wo_sb[:, 0, eo * NQ:(eo + 1) * NQ], start=True, stop=True)
                o0 = opool.tile([128, NQ], F32, tag=f"o0_{m}_{eo}", name="o0")
                nc.scalar.copy(out=o0, in_=po)
                o0s[(m, eo)] = o0

            def oproj_kp1(m, eo):
                po = ppool.tile([128, NQ], F32, tag="pp", name="po")
                nc.tensor.matmul(
                    po, ctxT[1][:, m * 128:(m + 1) * 128],
                    wo_sb[:, 1, eo * NQ:(eo + 1) * NQ], start=True, stop=True)
                o = opool.tile([128, NQ], F16, tag="o", name="o")
                nc.vector.tensor_tensor(out=o, in0=o0s[(m, eo)], in1=po,
                                        op=add)
                nc.sync.dma_start(
                    out=out_part[m * 128:(m + 1) * 128, eo * NQ:(eo + 1) * NQ],
                    in_=o)

            # ---- attention (per head-pair p, nq window of 512/256) ----
            # Row-packed S + dual-engine exp; see module docstring.
            # Two filler queues: `fillers` (projection groups; no dependence
            # on fresh ctxT -> drain eagerly) and `parcels` (out-proj; read
            # ctxT columns written by the PREVIOUS window's normalize chain
            # -> drain only from item 4 so the PE never head-of-line blocks
            # on that chain).
            from collections import deque
            fillers = deque()
            parcels = deque()
            nfill = deque()  # deferred normalize steps of the previous window

            def K0(n):
                return lambda: kq_group(0, n, wk_sb, bk_sb, kT)

            def Q0(n):
                return lambda: kq_group(0, n, wq_sb, bq_sb, qT)

            def K1(n):
                return lambda: kq_group(1, n, wk_sb, bk_sb, kT)

            def Q1(n):
                return lambda: kq_group(1, n, wq_sb, bq_sb, qT)

            def V(n, tl):
                return lambda: v_group(n, tl)

            wo_dma = lambda: nc.sync.dma_start(  # noqa: E731
                out=wo_sb, in_=woT.rearrange("(k p) e -> p k e", p=128))

            # per-window static filler lists (window key = (p, index))
            sched = {
                (0, 0): ([V(0, 2), V(0, 3), K0(1)]
                         + [V(1, tl) for tl in range(4)] + [K0(2)]
                         + [V(2, tl) for tl in range(4)] + [K0(3)]
                         + [V(3, tl) for tl in range(2)] + [Q0(1)]
                         + [V(3, tl) for tl in range(2, 4)]),
                (0, 1): [Q0(2), K1(0), Q1(0)],
                (0, 2): [Q0(3), K1(1), wo_dma],
                (0, 3): [K1(2)],
                (1, 0): [K1(3), Q1(1)],
                (1, 1): [Q1(2)],
                (1, 2): [Q1(3)],
                (1, 3): [lambda m=m, eo=eo: oproj_kp0(m, eo)
                         for m in range(14, 16) for eo in range(2)],
            }

            for p in range(2):
                if p == 0:
                    wins = [(jq * NQA, NQA) for jq in range(NJA)]
                else:
                    # final window split in two 256-halves: the serial tail
                    # (normalize chain + kp1 finishers + stores) covers only
                    # 2 m-chunks
                    wins = [(jq * NQA, NQA) for jq in range(NJA - 1)]
                    wins += [(N - NQA, NQA // 2), (N - NQA // 2, NQA // 2)]
                for jw, (q0, w) in enumerate(wins):
                    statics = sched.get((p, jw), [])
                    fillers.extendleft(reversed(statics))
                    n_static = len(statics)
                    cps = [cpool.tile([HD + 1, w], F32, tag=f"c{h}",
                                      name=f"c{h}") for h in range(2)]

                    def s_mms_pair(g, q0=q0, w=w):
                        # both heads' S tiles, head-interleaved so adjacent
                        # matmuls hit disjoint row groups and run concurrent
                        sps = [spool.tile([128, GK, w], F32,
                                          tag=f"s{h}", name=f"s{h}")
                               for h in range(2)]
                        for i, t in enumerate(g):
                            for h in range(2):
                                lo = 64 * h
                                nc.tensor.matmul(
                                    sps[h][:, i, :],
                                    kT[p][lo:lo + 64, t * 128:(t + 1) * 128],
                                    qT[p][lo:lo + 64, q0:q0 + w],
                                    start=True, stop=True)
                        return sps

                    def exp_acts(sps, wi, w=w):
                        # one head's exp on ACT (true), the other on DVE
                        # (Schraudolph); roles alternate per item
                        ha = wi % 2
                        hd = 1 - ha
                        ea = epool.tile([128, GK, w], F16, tag="ea", name="ea")
                        nc.scalar.activation(ea, sps[ha], FP.Exp, scale=SCALE)
                        ed = epool.tile([128, GK, w], I16, tag="ed", name="ed")
                        nc.vector._custom_dve(EXP_OP, out=ed, in0=sps[hd],
                                              s0=A_EXP, s1=B_EXP)
                        es = [None, None]
                        es[ha] = ea
                        es[hd] = ed
                        return es

                    def ctx_mms(e, g, h):
                        hloc = 2 * p + h
                        for i, t in enumerate(g):
                            src = e[:, i, :]
                            if e.dtype == I16:
                                src = src.bitcast(F16)
                            nc.tensor.matmul(
                                cps[h],
                                v_all[:, t, hloc * 65:(hloc + 1) * 65],
                                src,
                                start=(t == 0), stop=(t == NT - 1))

                    work = [tuple(range(gi * GK, (gi + 1) * GK))
                            for gi in range(NG)]
                    n_pop = len(fillers)  # drain this window's statics fully
                    n_parcel = len(parcels)
                    popped = ppopped = 0
                    pend = deque()  # ctx lags TWO items behind S/exp
                    for wi, g in enumerate(work):
                        # order within an item: ctx(u-2) FIRST (its e tiles
                        # are certainly ready, so the PE always has ready
                        # work in front of S(u)'s s-tag wait), then S(u),
                        # exps(u), deferred-normalize step, fillers.
                        if len(pend) == 2:
                            es, gp = pend.popleft()
                            for h in range(2):
                                ctx_mms(es[h], gp, h)
                        sps = s_mms_pair(g)
                        pend.append((exp_acts(sps, wi), g))
                        if wi >= 1 and nfill:
                            nfill.popleft()()
                        target = max(min(2 * (wi + 1), n_static),
                                     (wi + 1) * n_pop // max(1, len(work) - 1))
                        while fillers and popped < min(n_pop, target):
                            fillers.popleft()()
                            popped += 1
                        if wi >= 5:
                            ptarget = (wi - 4) * n_parcel // (len(work) - 5)
                            while parcels and ppopped < min(n_parcel, ptarget):
                                parcels.popleft()()
                                ppopped += 1
                    while pend:
                        es, gp = pend.popleft()
                        for h in range(2):
                            ctx_mms(es[h], gp, h)
                    while fillers and popped < n_pop:
                        fillers.popleft()()
                        popped += 1
                    while parcels and ppopped < n_parcel:
                        parcels.popleft()()
                        ppopped += 1

                    # normalize: ctx^T[0:64] * (1 / rowsum); rowsum in row 64.
                    # rowsum copy + ctx staging run now (release the c PSUM
                    # banks); the chain broadcast -> reciprocal -> multiply
                    # is DEFERRED into items 1-3 of the NEXT window so the
                    # gpsimd wait never head-of-line-blocks the DVE exp
                    # stream at the window boundary. Last window runs the
                    # chain inline (straight from PSUM; gates only the tail).
                    last = (p == 1 and jw == len(wins) - 1)
                    rss = [None, None]
                    css = [None, None]
                    for h in range(2):
                        rs = npool.tile([1, w], F32, tag="rs", name="rs")
                        nc.vector.tensor_copy(rs, cps[h][64:65, :])
                        rss[h] = rs
                        if not last:
                            # stage ctx to SBUF to release the c PSUM bank
                            # for the next window; in p=1 windows ACT paces
                            # the loop (exps + o-copies) so use DVE there
                            cs = npool.tile([64, w], F32, tag="cs", name="cs")
                            if p == 0:
                                nc.scalar.copy(out=cs, in_=cps[h][0:64, :])
                            else:
                                nc.vector.tensor_copy(out=cs, in_=cps[h][0:64, :])
                            css[h] = cs

                    def n_bcast(rss=rss, w=w):
                        rbs = []
                        for h in range(2):
                            rb = npool.tile([64, w], F32, tag=f"rb{h}",
                                            name="rb")
                            nc.gpsimd.partition_broadcast(rb, rss[h])
                            rbs.append(rb)
                        return rbs

                    def n_recip(rbs, w=w):
                        rcs = []
                        for h in range(2):
                            rc = npool.tile([64, w], F32, tag=f"rc{h}",
                                            name="rc")
                            nc.vector.reciprocal_approx_fast(out=rc, in_=rbs[h])
                            rcs.append(rc)
                        return rcs

                    def n_mult(rcs, css=css, p=p, q0=q0, w=w):
                        for h in range(2):
                            nc.vector.tensor_tensor(
                                out=ctxT[p][h * 64:(h + 1) * 64, q0:q0 + w],
                                in0=css[h], in1=rcs[h], op=mult)

                    if last:
                        rbs = n_bcast()
                        rcs = n_recip(rbs)
                        for h in range(2):
                            nc.vector.tensor_tensor(
                                out=ctxT[p][h * 64:(h + 1) * 64, q0:q0 + w],
                                in0=cps[h][0:64, :], in1=rcs[h], op=mult)
                    else:
                        box = {}
                        nfill.append(lambda box=box: box.__setitem__(
                            "rbs", n_bcast()))
                        nfill.append(lambda box=box: box.__setitem__(
                            "rcs", n_recip(box["rbs"])))
                        nfill.append(lambda box=box: n_mult(box["rcs"]))
                    if p == 1:
                        # ctxT1 columns for this window are final -> out-proj
                        # parcels for the covered m-chunks can run (delayed
                        # to item>=4 of the next window by the parcel queue)
                        for m in range(q0 // 128, (q0 + w) // 128):
                            for eo in range(2):
                                f = out_proj_parcel if jw < len(wins) - 1 else oproj_kp1
                                parcels.append(lambda m=m, eo=eo, f=f: f(m, eo))
            while nfill or fillers or parcels:
                q = nfill if nfill else (fillers if fillers else parcels)
                q.popleft()()

    nc.compile()
    return nc


_NC_CACHE = {}


def _get_program():
    if "nc" not in _NC_CACHE:
        _NC_CACHE["nc"] = build_program()
    return _NC_CACHE["nc"]


def make_in_maps(x, Wq, bq, Wk, bk, Wv, bv, Wo, bo):
    x = np.asarray(x)
    xTs = [np.ascontiguousarray(x[b].T.astype(np.float16)) for b in range(B)]
    in_maps = []
    for c in range(N_CORES):
        b, hg = divmod(c, TP)
        sl = slice(hg * DQ, (hg + 1) * DQ)
        in_maps.append({
            "xT": xTs[b],
            "wqT": np.ascontiguousarray(np.asarray(Wq, np.float16)[sl, :].T),
            "wkT": np.ascontiguousarray(np.asarray(Wk, np.float16)[sl, :].T),
            "wvT": np.ascontiguousarray(np.asarray(Wv, np.float16)[sl, :].T),
            "woT": np.ascontiguousarray(np.asarray(Wo, np.float16)[:, sl].T),
            "bq_s": np.ascontiguousarray(np.asarray(bq, np.float32)[sl]),
            "bk_s": np.ascontiguousarray(np.asarray(bk, np.float32)[sl]),
        })
    return in_maps


def assemble_output(results, Wv_bias_term):
    out = np.empty((B, N, EMB), np.float32)
    for b in range(B):
        acc = results[b * TP]["out_part"].astype(np.float32)
        for g in range(1, TP):
            acc = acc + results[b * TP + g]["out_part"]
        out[b] = acc + Wv_bias_term
    return out


def kernel(x, Wq, bq, Wk, bk, Wv, bv, Wo, bo):
    nc = _get_program()
    in_maps = make_in_maps(x, Wq, bq, Wk, bk, Wv, bv, Wo, bo)
    res = run_bass_kernel_spmd(nc, in_maps, list(range(N_CORES)))
    bias_term = (np.asarray(bv, np.float32) @ np.asarray(Wo, np.float32).T
                 + np.asarray(bo, np.float32))
    return assemble_output(res.results, bias_term)


# revision 20
# speedup vs baseline: 1.0074x; 1.0074x over previous
"""Multi-head self-attention (b=2, n=2048, emb=1024, heads=16) on 8 trn2 cores.

Sharding: core c = (b, hg) with b = c // 4, hg = c % 4. Data parallel over
batch, tensor parallel over head-groups (4 heads / 256 emb-cols per core).
Each core computes Q/K/V projections for its heads, full attention for its
heads, and a partial output projection ctx_hg @ Wo[:, hg_slice].T of shape
[2048, 1024]. The host sums the 4 partials per batch (Megatron row-parallel
reduce done on host) and adds the rank-1 bias term bv @ Wo.T + bo.

Key device-side structure (v2):
- x^T resident in SBUF; fp16 matmuls everywhere; q/k biases fused into the
  PSUM->SBUF copies; v/o biases are the host-side rank-1 term.
- S matmuls are ROW-PACKED: the two heads of a pair live in partition rows
  0:64 / 64:128 of qT/kT, and their S matmuls are emitted head-interleaved
  so adjacent matmuls target disjoint 64-row groups of the PE array
  (tile_position auto-derived from base_partition) and stream concurrently
  -> a packed pair costs ~512 cycles instead of 2x512.
- The two heads' exps run on DIFFERENT engines concurrently: one on ACT
  (true exp, 1024-wide out of PSUM) and one on DVE via a custom
  Schraudolph op: e = bitcast_fp16(int16(relu(S*(scale*1024/ln2)+15315.3)))
  (max rel err ~3%, mean ~1.8%, HW-validated). Which head uses which
  engine alternates per work item, so the approx noise averages across
  nk-chunks inside every head instead of concentrating in one head
  (absmax-rel error ~2.4e-3 in numpy sim). This halves the exp-chain
  period AND lets both S psum tags recycle each item.
- V carries a ones column per head -> ctx matmul row 64 accumulates the
  softmax denominators for free; normalize = rowsum -> partition 0 (DVE),
  gpsimd broadcast, reciprocal_approx_fast (DVE), multiply (gpsimd from the
  staged SBUF copy; last window straight from PSUM on DVE to shorten the
  tail chain).
- Engine balance: out-proj PSUM->SBUF copies run on ACT (scalar.copy),
  normalize multiplies on gpsimd, kq-bias adds + v copies + exp(DVE half)
  + reciprocal on DVE; out_part stores issue on the sync DMA queue.
- Projection/out-proj parcels fill PE slack inside the attention windows
  from per-window filler lists (emission-deadline ordered, paced ~2/item).
"""

import os
import sys

for _p in ("/opt/trn_rl_repo", "/root/.axon_site/_ro/trn_rl_repo"):
    if os.path.isdir(_p) and _p not in sys.path:
        sys.path.append(_p)

import numpy as np

import concourse.bass as bass  # noqa: F401  (engine types pulled via nc)
import concourse.mybir as mybir
import concourse.tile as tile
from concourse import bacc
from concourse.bass_utils import run_bass_kernel_spmd
from concourse import dve_ops
from concourse.dve_spec import Spec, Src0, C0, C1, relu, lower
from concourse.dve_uop import DveOpSpec

B, N, EMB, HEADS, HD = 2, 2048, 1024, 16, 64
N_CORES = 8
TP = 4                      # head-group shards per batch
DQ = EMB // TP              # 256 emb-cols (4 heads) per core
SCALE = HD ** -0.5          # 0.125

F32 = mybir.dt.float32
F16 = mybir.dt.float16
I16 = mybir.dt.int16
FP = mybir.ActivationFunctionType

NQ = 512                    # nq chunk for projections / out-proj
NJ = N // NQ                # 4 nq chunks
NQA = 512                   # nq chunk for attention
NJA = N // NQA              # 4 attention nq chunks
NKC = 128                   # nk chunk (ctx contraction)
NT = N // NKC               # 16 nk chunks
KC = EMB // 128             # 8 e chunks
GK = 2                      # nk chunks per S-psum group (1024-wide exps)
NG = NT // GK               # 8 groups per (head-pair, j)

# Schraudolph fp16-bit exp: bits = relu(x*SCALE*1024/ln2 + (15360-44.7))
A_EXP = float(SCALE * 1024.0 / np.log(2.0))
B_EXP = 15360.0 - 44.7


def _register_exp_op():
    for op in dve_ops.OPS:
        if op.name == "EXP_SCHRAUDOLPH_ANT":
            return op

    def _ref(in0, in1, c0, c1, c2):
        return np.maximum(in0 * c0 + c1, 0.0)

    spec = Spec(body=relu(Src0 * C0 + C1), reference=_ref)
    shas = {}
    for ver in ("v3", "v4"):
        tmp = DveOpSpec(name="EXP_SCHRAUDOLPH_ANT", opcode=1,
                        uops=lower(spec, ver=ver), rd1_en=False)
        shas[ver] = tmp.sha(ver)
    op = dve_ops.DveOp("EXP_SCHRAUDOLPH_ANT", spec, subdim=False, uops_sha=shas)
    dve_ops.OPS.append(op)
    dve_ops.CUSTOM_DVE_SPECS[op.name] = op.spec
    dve_ops._SUB_OPCODE_FOR_NAME[op.name] = (
        dve_ops._CUSTOM_DVE_ROW_BASE + len(dve_ops.OPS) - 1)
    return op


EXP_OP = _register_exp_op()


def build_program():
    """Build + compile the single SPMD program all 8 cores run."""
    nc = bacc.Bacc("TRN2", target_bir_lowering=False, debug=False,
                   num_devices=N_CORES)

    xT = nc.dram_tensor("xT", [EMB, N], F16, kind="ExternalInput").ap()
    wqT = nc.dram_tensor("wqT", [EMB, DQ], F16, kind="ExternalInput").ap()
    wkT = nc.dram_tensor("wkT", [EMB, DQ], F16, kind="ExternalInput").ap()
    wvT = nc.dram_tensor("wvT", [EMB, DQ], F16, kind="ExternalInput").ap()
    woT = nc.dram_tensor("woT", [DQ, EMB], F16, kind="ExternalInput").ap()
    bqd = nc.dram_tensor("bq_s", [DQ], F32, kind="ExternalInput").ap()
    bkd = nc.dram_tensor("bk_s", [DQ], F32, kind="ExternalInput").ap()
    # fp16 partials: host sums 4 of them in fp32; quantization of the
    # partial (|.| ~ 1, ulp ~ 1e-3) adds ~1e-3 absmax-relative error --
    # well under the 2e-2 gate -- and halves the store traffic + tail.
    out_part = nc.dram_tensor("out_part", [N, EMB], F16,
                              kind="ExternalOutput").ap()

    with tile.TileContext(nc) as tc:
        with (
            tc.tile_pool(name="const", bufs=1) as const,
            tc.tile_pool(name="persist", bufs=1) as persist,
            tc.tile_pool(name="epool", bufs=3) as epool,
            tc.tile_pool(name="npool", bufs=2) as npool,
            tc.tile_pool(name="opool", bufs=4) as opool,
            # PSUM static budget (8 banks): pp 2 + s0 2 + s1 2 + c0 1 + c1 1
            tc.tile_pool(name="ppool", bufs=2, space="PSUM") as ppool,
            tc.tile_pool(name="spool", bufs=1, space="PSUM") as spool,
            tc.tile_pool(name="cpool", bufs=1, space="PSUM") as cpool,
        ):
            # ---- constants + resident x ----
            # first projection group needs wk chunks + x n-slice 0 only; those
            # DMAs go first, fine-grained, spread across the Sync / GpSimd /
            # Scalar queues so no single sequencer's DGE time serializes the
            # startup.
            wq_sb = const.tile([128, KC, DQ], F16, tag="wq")
            wk_sb = const.tile([128, KC, DQ], F16, tag="wk")
            wv_sb = const.tile([128, KC, DQ], F16, tag="wv")
            x_sb = const.tile([128, KC, N], F16, tag="x")
            xTr = xT.rearrange("(k p) n -> k p n", p=128)
            bk_sb = const.tile([128, 2], F32, tag="bk")
            bq_sb = const.tile([128, 2], F32, tag="bq")
            # Wave plan (3 DGE queues: sync/scalar/gpsimd, ~0.6us/desc):
            # wave 1 carries the first kq_group's needs (wk + x0) plus wq
            # (needed right after on PE) in per-chunk arrival order; wv and
            # the later x slices follow before their first consumers.
            nc.scalar.dma_start(out=bk_sb, in_=bkd.rearrange("(m p) -> p m", p=128))
            nc.scalar.dma_start(out=bq_sb, in_=bqd.rearrange("(m p) -> p m", p=128))
            wkr = wkT.rearrange("(k p) d -> k p d", p=128)
            wqr = wqT.rearrange("(k p) d -> k p d", p=128)
            wvr = wvT.rearrange("(k p) d -> k p d", p=128)
            for k in range(KC):
                nc.sync.dma_start(out=wk_sb[:, k, :], in_=wkr[k])
                nc.gpsimd.dma_start(out=x_sb[:, k, 0:NQ], in_=xTr[k, :, 0:NQ])
                nc.scalar.dma_start(out=wq_sb[:, k, :], in_=wqr[k])
            for k in range(KC):
                nc.sync.dma_start(out=x_sb[:, k, NQ:2 * NQ],
                                  in_=xTr[k, :, NQ:2 * NQ])
                nc.gpsimd.dma_start(out=wv_sb[:, k, :], in_=wvr[k])
                nc.scalar.dma_start(out=x_sb[:, k, 2 * NQ:3 * NQ],
                                    in_=xTr[k, :, 2 * NQ:3 * NQ])
            for k in range(KC):
                nc.gpsimd.dma_start(out=x_sb[:, k, 3 * NQ:N],
                                    in_=xTr[k, :, 3 * NQ:N])
            # wo is needed only by the out-projection (~100us in) -- deferred
            # into the filler stream to keep startup queues clear
            wo_sb = const.tile([128, 2, EMB], F16, tag="wo")

            # ---- persistent activations ----
            qT = [persist.tile([128, N], F16, tag=f"qT{p}", name=f"qT{p}") for p in range(2)]
            kT = [persist.tile([128, N], F16, tag=f"kT{p}", name=f"kT{p}") for p in range(2)]
            ctxT = [persist.tile([128, N], F16, tag=f"ctxT{p}", name=f"ctxT{p}") for p in range(2)]
            # V for all 4 local heads: [nk-part, t, head*65 + (0:64 | ones)]
            v_all = persist.tile([128, NT, 4 * (HD + 1)], F16, tag="v")
            for h in range(4):
                nc.vector.memset(v_all[:, :, h * 65 + 64], 1.0)

            add, mult = mybir.AluOpType.add, mybir.AluOpType.mult

            # ---- projection building blocks ----
            # Each is one PSUM accumulation group on the double-buffered pp
            # tag, small enough to slot between attention groups.
            def kq_group(p, n, wsb, bsb, dst):
                ps = ppool.tile([128, NQ], F32, tag="pp", name="kqp")
                for k in range(KC):
                    nc.tensor.matmul(
                        ps, wsb[:, k, p * 128:(p + 1) * 128],
                        x_sb[:, k, n * NQ:(n + 1) * NQ],
                        start=(k == 0), stop=(k == KC - 1))
                nc.vector.tensor_tensor(
                    out=dst[p][:, n * NQ:(n + 1) * NQ], in0=ps,
                    in1=bsb[:, p:p + 1].broadcast_to([128, NQ]), op=add)

            def v_group(n, tl):
                # V for ALL 4 local heads at once (256-col moving wv)
                t = n * 4 + tl
                ps = ppool.tile([128, DQ], F32, tag="pp", name="vp")
                for k in range(KC):
                    nc.tensor.matmul(
                        ps, x_sb[:, k, t * 128:(t + 1) * 128],
                        wv_sb[:, k, :],
                        start=(k == 0), stop=(k == KC - 1))
                vv = v_all[:, t, :].rearrange("p (h c) -> p h c", c=65)
                nc.vector.tensor_copy(
                    out=vv[:, :, 0:64],
                    in_=ps.rearrange("p (h c) -> p h c", c=64))

            # Minimal cold prefix: only the work attention j=0 strictly
            # needs before its first items runs up front -- K(n=0), Q(n=0),
            # V(t=0,1). Everything else streams through the hot attention
            # windows as fillers.
            kq_group(0, 0, wk_sb, bk_sb, kT)
            kq_group(0, 0, wq_sb, bq_sb, qT)
            v_group(0, 0)
            v_group(0, 1)

            # ---- out-projection ----
            # one (m, eo) parcel: both head-pair passes accumulate into a
            # single pp-tag PSUM group, then one ACT copy to SBUF + store.
            def out_proj_parcel(m, eo):
                po = ppool.tile([128, NQ], F32, tag="pp", name="po")
                for kp in range(2):
                    nc.tensor.matmul(
                        po, ctxT[kp][:, m * 128:(m + 1) * 128],
                        wo_sb[:, kp, eo * NQ:(eo + 1) * NQ],
                        start=(kp == 0), stop=(kp == 1))
                o = opool.tile([128, NQ], F16, tag="o", name="o")
                nc.scalar.copy(out=o, in_=po)
                nc.sync.dma_start(
                    out=out_part[m * 128:(m + 1) * 128, eo * NQ:(eo + 1) * NQ],
                    in_=o)

            # The LAST window's m-chunks split the parcel: the ctxT0 half
            # runs during the window (ctxT0 is final since p0), leaving only
            # one matmul + add + store per parcel on the serial tail.
            o0s = {}

            def oproj_kp0(m, eo):
                po = ppool.tile([128, NQ], F32, tag="pp", name="po")
                nc.tensor.matmul(
                    po, ctxT[0][:, m * 128:(m + 1) * 128],
                    wo_sb[:, 0, eo * NQ:(eo + 1) * NQ], start=True, stop=True)
                o0 = opool.tile([128, NQ], F32, tag=f"o0_{m}_{eo}", name="o0")
                nc.scalar.copy(out=o0, in_=po)
                o0s[(m, eo)] = o0

            def oproj_kp1(m, eo):
                po = ppool.tile([128, NQ], F32, tag="pp", name="po")
                nc.tensor.matmul(
                    po, ctxT[1][:, m * 128:(m + 1) * 128],
                    wo_sb[:, 1, eo * NQ:(eo + 1) * NQ], start=True, stop=True)
                o = opool.tile([128, NQ], F16, tag="o", name="o")
                nc.vector.tensor_tensor(out=o, in0=o0s[(m, eo)], in1=po,
                                        op=add)
                nc.sync.dma_start(
                    out=out_part[m * 128:(m + 1) * 128, eo * NQ:(eo + 1) * NQ],
                    in_=o)

            # ---- attention (per head-pair p, nq window of 512/256) ----
            # Row-packed S + dual-engine exp; see module docstring.
            # Two filler queues: `fillers` (projection groups; no dependence
            # on fresh ctxT -> drain eagerly) and `parcels` (out-proj; read
            # ctxT columns written by the PREVIOUS window's normalize chain
            # -> drain only from item 4 so the PE never head-of-line blocks
            # on that chain).
            from collections import deque
            fillers = deque()
            parcels = deque()
            nfill = deque()  # deferred normalize steps of the previous window

            def K0(n):
                return lambda: kq_group(0, n, wk_sb, bk_sb, kT)

            def Q0(n):
                return lambda: kq_group(0, n, wq_sb, bq_sb, qT)

            def K1(n):
                return lambda: kq_group(1, n, wk_sb, bk_sb, kT)

            def Q1(n):
                return lambda: kq_group(1, n, wq_sb, bq_sb, qT)

            def V(n, tl):
                return lambda: v_group(n, tl)

            wo_dma = lambda: nc.sync.dma_start(  # noqa: E731
                out=wo_sb, in_=woT.rearrange("(k p) e -> p k e", p=128))

            # per-window static filler lists (window key = (p, index))
            sched = {
                (0, 0): ([V(0, 2), V(0, 3), K0(1)]
                         + [V(1, tl) for tl in range(4)] + [K0(2)]
                         + [V(2, tl) for tl in range(4)] + [K0(3)]
                         + [V(3, tl) for tl in range(2)] + [Q0(1)]
                         + [V(3, tl) for tl in range(2, 4)]),
                (0, 1): [Q0(2), K1(0), Q1(0)],
                (0, 2): [Q0(3), K1(1), wo_dma],
                (0, 3): [K1(2)],
                (1, 0): [K1(3), Q1(1)],
                (1, 1): [Q1(2)],
                (1, 2): [Q1(3)],
                (1, 3): [lambda m=m, eo=eo: oproj_kp0(m, eo)
                         for m in range(14, 16) for eo in range(2)],
            }

            for p in range(2):
                if p == 0:
                    wins = [(jq * NQA, NQA) for jq in range(NJA)]
                else:
                    # final window split in two 256-halves: the serial tail
                    # (normalize chain + kp1 finishers + stores) covers only
                    # 2 m-chunks
                    wins = [(jq * NQA, NQA) for jq in range(NJA - 1)]
                    wins += [(N - NQA, NQA // 2), (N - NQA // 2, NQA // 2)]
                for jw, (q0, w) in enumerate(wins):
                    statics = sched.get((p, jw), [])
                    fillers.extendleft(reversed(statics))
                    n_static = len(statics)
                    cps = [cpool.tile([HD + 1, w], F32, tag=f"c{h}",
                                      name=f"c{h}") for h in range(2)]

                    def s_mms_pair(g, q0=q0, w=w):
                        # both heads' S tiles, head-interleaved so adjacent
                        # matmuls hit disjoint row groups and run concurrent
                        sps = [spool.tile([128, GK, w], F32,
                                          tag=f"s{h}", name=f"s{h}")
                               for h in range(2)]
                        for i, t in enumerate(g):
                            for h in range(2):
                                lo = 64 * h
                                nc.tensor.matmul(
                                    sps[h][:, i, :],
                                    kT[p][lo:lo + 64, t * 128:(t + 1) * 128],
                                    qT[p][lo:lo + 64, q0:q0 + w],
                                    start=True, stop=True)
                        return sps

                    def exp_acts(sps, wi, w=w):
                        # one head's exp on ACT (true), the other on DVE
                        # (Schraudolph); roles alternate per item
                        ha = wi % 2
                        hd = 1 - ha
                        ea = epool.tile([128, GK, w], F16, tag="ea", name="ea")
                        nc.scalar.activation(ea, sps[ha], FP.Exp, scale=SCALE)
                        ed = epool.tile([128, GK, w], I16, tag="ed", name="ed")
                        nc.vector._custom_dve(EXP_OP, out=ed, in0=sps[hd],
                                              s0=A_EXP, s1=B_EXP)
                        es = [None, None]
                        es[ha] = ea
                        es[hd] = ed
                        return es

                    def ctx_mms(e, g, h):
                        hloc = 2 * p + h
                        for i, t in enumerate(g):
                            src = e[:, i, :]
                            if e.dtype == I16:
                                src = src.bitcast(F16)
                            nc.tensor.matmul(
                                cps[h],
                                v_all[:, t, hloc * 65:(hloc + 1) * 65],
                                src,
                                start=(t == 0), stop=(t == NT - 1))

                    work = [tuple(range(gi * GK, (gi + 1) * GK))
                            for gi in range(NG)]
                    n_pop = len(fillers)  # drain this window's statics fully
                    n_parcel = len(parcels)
                    popped = ppopped = 0
                    pend = deque()  # ctx lags TWO items behind S/exp
                    for wi, g in enumerate(work):
                        # order within an item: ctx(u-2) FIRST (its e tiles
                        # are certainly ready, so the PE always has ready
                        # work in front of S(u)'s s-tag wait), then S(u),
                        # exps(u), deferred-normalize step, fillers.
                        if len(pend) == 2:
                            es, gp = pend.popleft()
                            for h in range(2):
                                ctx_mms(es[h], gp, h)
                        sps = s_mms_pair(g)
                        pend.append((exp_acts(sps, wi), g))
                        if wi >= 1 and nfill:
                            nfill.popleft()()
                        target = max(min(2 * (wi + 1), n_static),
                                     (wi + 1) * n_pop // max(1, len(work) - 1))
                        while fillers and popped < min(n_pop, target):
                            fillers.popleft()()
                            popped += 1
                        if wi >= 5:
                            ptarget = (wi - 4) * n_parcel // (len(work) - 5)
                            while parcels and ppopped < min(n_parcel, ptarget):
                                parcels.popleft()()
                                ppopped += 1
                    while pend:
                        es, gp = pend.popleft()
                        for h in range(2):
                            ctx_mms(es[h], gp, h)
                    while fillers and popped < n_pop:
                        fillers.popleft()()
                        popped += 1
                    while parcels and ppopped < n_parcel:
                        parcels.popleft()()
                        ppopped += 1

                    # normalize: ctx^T[0:64] * (1 / rowsum); rowsum in row 64.
                    # rowsum copy + ctx staging run now (release the c PSUM
                    # banks); the chain broadcast -> reciprocal -> multiply
                    # is DEFERRED into items 1-3 of the NEXT window so the
                    # gpsimd wait never head-of-line-blocks the DVE exp
                    # stream at the window boundary. Last window runs the
                    # chain inline (straight from PSUM; gates only the tail).
                    last = (p == 1 and jw == len(wins) - 1)
                    rss = [None, None]
                    css = [None, None]
                    for h in range(2):
                        rs = npool.tile([1, w], F32, tag="rs", name="rs")
                        nc.vector.tensor_copy(rs, cps[h][64:65, :])
                        rss[h] = rs
                        if not last:
                            # stage ctx to SBUF (ACT: the DVE queue is the
                            # exp critical path) to release the c PSUM bank
                            # for the next window
                            cs = npool.tile([64, w], F32, tag="cs", name="cs")
                            nc.scalar.copy(out=cs, in_=cps[h][0:64, :])
                            css[h] = cs

                    def n_bcast(rss=rss, w=w):
                        rbs = []
                        for h in range(2):
                            rb = npool.tile([64, w], F32, tag=f"rb{h}",
                                            name="rb")
                            nc.gpsimd.partition_broadcast(rb, rss[h])
                            rbs.append(rb)
                        return rbs

                    def n_recip(rbs, w=w):
                        rcs = []
                        for h in range(2):
                            rc = npool.tile([64, w], F32, tag=f"rc{h}",
                                            name="rc")
                            nc.vector.reciprocal_approx_fast(out=rc, in_=rbs[h])
                            rcs.append(rc)
                        return rcs

                    def n_mult(rcs, css=css, p=p, q0=q0, w=w):
                        for h in range(2):
                            nc.vector.tensor_tensor(
                                out=ctxT[p][h * 64:(h + 1) * 64, q0:q0 + w],
                                in0=css[h], in1=rcs[h], op=mult)

                    if last:
                        rbs = n_bcast()
                        rcs = n_recip(rbs)
                        for h in range(2):
                            nc.vector.tensor_tensor(
                                out=ctxT[p][h * 64:(h + 1) * 64, q0:q0 + w],
                                in0=cps[h][0:64, :], in1=rcs[h], op=mult)
                    else:
                        box = {}
                        nfill.append(lambda box=box: box.__setitem__(
                            "rbs", n_bcast()))
                        nfill.append(lambda box=box: box.__setitem__(
                            "rcs", n_recip(box["rbs"])))
                        nfill.append(lambda box=box: n_mult(box["rcs"]))
                    if p == 1:
                        # ctxT1 columns for this window are final -> out-proj
                        # parcels for the covered m-chunks can run (delayed
                        # to item>=4 of the next window by the parcel queue)
                        for m in range(q0 // 128, (q0 + w) // 128):
                            for eo in range(2):
                                f = out_proj_parcel if jw < len(wins) - 1 else oproj_kp1
                                parcels.append(lambda m=m, eo=eo, f=f: f(m, eo))
            while nfill or fillers or parcels:
                q = nfill if nfill else (fillers if fillers else parcels)
                q.popleft()()

    nc.compile()
    return nc


_NC_CACHE = {}


def _get_program():
    if "nc" not in _NC_CACHE:
        _NC_CACHE["nc"] = build_program()
    return _NC_CACHE["nc"]


def make_in_maps(x, Wq, bq, Wk, bk, Wv, bv, Wo, bo):
    x = np.asarray(x)
    xTs = [np.ascontiguousarray(x[b].T.astype(np.float16)) for b in range(B)]
    in_maps = []
    for c in range(N_CORES):
        b, hg = divmod(c, TP)
        sl = slice(hg * DQ, (hg + 1) * DQ)
        in_maps.append({
            "xT": xTs[b],
            "wqT": np.ascontiguousarray(np.asarray(Wq, np.float16)[sl, :].T),
            "wkT": np.ascontiguousarray(np.asarray(Wk, np.float16)[sl, :].T),
            "wvT": np.ascontiguousarray(np.asarray(Wv, np.float16)[sl, :].T),
            "woT": np.ascontiguousarray(np.asarray(Wo, np.float16)[:, sl].T),
            "bq_s": np.ascontiguousarray(np.asarray(bq, np.float32)[sl]),
            "bk_s": np.ascontiguousarray(np.asarray(bk, np.float32)[sl]),
        })
    return in_maps


def assemble_output(results, Wv_bias_term):
    out = np.empty((B, N, EMB), np.float32)
    for b in range(B):
        acc = results[b * TP]["out_part"].astype(np.float32)
        for g in range(1, TP):
            acc = acc + results[b * TP + g]["out_part"]
        out[b] = acc + Wv_bias_term
    return out


def kernel(x, Wq, bq, Wk, bk, Wv, bv, Wo, bo):
    nc = _get_program()
    in_maps = make_in_maps(x, Wq, bq, Wk, bk, Wv, bv, Wo, bo)
    res = run_bass_kernel_spmd(nc, in_maps, list(range(N_CORES)))
    bias_term = (np.asarray(bv, np.float32) @ np.asarray(Wo, np.float32).T
                 + np.asarray(bo, np.float32))
    return assemble_output(res.results, bias_term)


# revision 22
# speedup vs baseline: 1.0121x; 1.0047x over previous
"""Multi-head self-attention (b=2, n=2048, emb=1024, heads=16) on 8 trn2 cores.

Sharding: core c = (b, hg) with b = c // 4, hg = c % 4. Data parallel over
batch, tensor parallel over head-groups (4 heads / 256 emb-cols per core).
Each core computes Q/K/V projections for its heads, full attention for its
heads, and a partial output projection ctx_hg @ Wo[:, hg_slice].T of shape
[2048, 1024]. The host sums the 4 partials per batch (Megatron row-parallel
reduce done on host) and adds the rank-1 bias term bv @ Wo.T + bo.

Key device-side structure (v2):
- x^T resident in SBUF; fp16 matmuls everywhere; q/k biases fused into the
  PSUM->SBUF copies; v/o biases are the host-side rank-1 term.
- S matmuls are ROW-PACKED: the two heads of a pair live in partition rows
  0:64 / 64:128 of qT/kT, and their S matmuls are emitted head-interleaved
  so adjacent matmuls target disjoint 64-row groups of the PE array
  (tile_position auto-derived from base_partition) and stream concurrently
  -> a packed pair costs ~512 cycles instead of 2x512.
- The two heads' exps run on DIFFERENT engines concurrently: one on ACT
  (true exp, 1024-wide out of PSUM) and one on DVE via a custom
  Schraudolph op: e = bitcast_fp16(int16(relu(S*(scale*1024/ln2)+15315.3)))
  (max rel err ~3%, mean ~1.8%, HW-validated). Which head uses which
  engine alternates per work item, so the approx noise averages across
  nk-chunks inside every head instead of concentrating in one head
  (absmax-rel error ~2.4e-3 in numpy sim). This halves the exp-chain
  period AND lets both S psum tags recycle each item.
- ctx matmuls are emitted FIRST in each item, lagging TWO items behind
  their S/exp, so the in-order PE queue always holds ready work in front
  of S's psum-tag wait.
- V carries a ones column per head -> ctx matmul row 64 accumulates the
  softmax denominators for free. The normalize chain (rowsum -> gpsimd
  partition_broadcast -> reciprocal_approx_fast -> multiply, DVE ops) is
  DEFERRED into items 1-3 of the next window so its gpsimd wait never
  head-of-line-blocks the DVE exp stream at a window boundary; the ctx
  PSUM banks are released immediately via an ACT staging copy. The last
  window runs the chain inline straight from PSUM (shortest tail).
- Engine balance (learned the hard way: the DVE queue IS the exp critical
  path, keep auxiliary copies off it): ctx staging + out-proj PSUM->SBUF
  copies on ACT; kq-bias adds, v copies, reciprocal, normalize multiplies
  on DVE; out_part stores issue on the sync DMA queue.
- Projection/out-proj parcels fill PE slack inside the attention windows
  from per-window filler lists (emission-deadline ordered); out-proj
  parcels drain only from item 4 so they never wait on the deferred
  normalize of the previous window.
"""

import os
import sys

for _p in ("/opt/trn_rl_repo", "/root/.axon_site/_ro/trn_rl_repo"):
    if os.path.isdir(_p) and _p not in sys.path:
        sys.path.append(_p)

import numpy as np

import concourse.bass as bass  # noqa: F401  (engine types pulled via nc)
import concourse.mybir as mybir
import concourse.tile as tile
from concourse import bacc
from concourse.bass_utils import run_bass_kernel_spmd
from concourse import dve_ops
from concourse.dve_spec import Spec, Src0, C0, C1, relu, lower
from concourse.dve_uop import DveOpSpec

B, N, EMB, HEADS, HD = 2, 2048, 1024, 16, 64
N_CORES = 8
TP = 4                      # head-group shards per batch
DQ = EMB // TP              # 256 emb-cols (4 heads) per core
SCALE = HD ** -0.5          # 0.125

F32 = mybir.dt.float32
F16 = mybir.dt.float16
I16 = mybir.dt.int16
FP = mybir.ActivationFunctionType

NQ = 512                    # nq chunk for projections / out-proj
NJ = N // NQ                # 4 nq chunks
NQA = 512                   # nq chunk for attention
NJA = N // NQA              # 4 attention nq chunks
NKC = 128                   # nk chunk (ctx contraction)
NT = N // NKC               # 16 nk chunks
KC = EMB // 128             # 8 e chunks
GK = 2                      # nk chunks per S-psum group (1024-wide exps)
NG = NT // GK               # 8 groups per (head-pair, j)

# Schraudolph fp16-bit exp: bits = relu(x*SCALE*1024/ln2 + (15360-44.7))
A_EXP = float(SCALE * 1024.0 / np.log(2.0))
B_EXP = 15360.0 - 44.7


def _register_exp_op():
    for op in dve_ops.OPS:
        if op.name == "EXP_SCHRAUDOLPH_ANT":
            return op

    def _ref(in0, in1, c0, c1, c2):
        return np.maximum(in0 * c0 + c1, 0.0)

    spec = Spec(body=relu(Src0 * C0 + C1), reference=_ref)
    shas = {}
    for ver in ("v3", "v4"):
        tmp = DveOpSpec(name="EXP_SCHRAUDOLPH_ANT", opcode=1,
                        uops=lower(spec, ver=ver), rd1_en=False)
        shas[ver] = tmp.sha(ver)
    op = dve_ops.DveOp("EXP_SCHRAUDOLPH_ANT", spec, subdim=False, uops_sha=shas)
    dve_ops.OPS.append(op)
    dve_ops.CUSTOM_DVE_SPECS[op.name] = op.spec
    dve_ops._SUB_OPCODE_FOR_NAME[op.name] = (
        dve_ops._CUSTOM_DVE_ROW_BASE + len(dve_ops.OPS) - 1)
    return op


EXP_OP = _register_exp_op()


def build_program():
    """Build + compile the single SPMD program all 8 cores run."""
    nc = bacc.Bacc("TRN2", target_bir_lowering=False, debug=False,
                   num_devices=N_CORES)

    xT = nc.dram_tensor("xT", [EMB, N], F16, kind="ExternalInput").ap()
    wqT = nc.dram_tensor("wqT", [EMB, DQ], F16, kind="ExternalInput").ap()
    wkT = nc.dram_tensor("wkT", [EMB, DQ], F16, kind="ExternalInput").ap()
    wvT = nc.dram_tensor("wvT", [EMB, DQ], F16, kind="ExternalInput").ap()
    woT = nc.dram_tensor("woT", [DQ, EMB], F16, kind="ExternalInput").ap()
    bqd = nc.dram_tensor("bq_s", [DQ], F32, kind="ExternalInput").ap()
    bkd = nc.dram_tensor("bk_s", [DQ], F32, kind="ExternalInput").ap()
    # fp16 partials: host sums 4 of them in fp32; quantization of the
    # partial (|.| ~ 1, ulp ~ 1e-3) adds ~1e-3 absmax-relative error --
    # well under the 2e-2 gate -- and halves the store traffic + tail.
    out_part = nc.dram_tensor("out_part", [N, EMB], F16,
                              kind="ExternalOutput").ap()

    with tile.TileContext(nc) as tc:
        with (
            tc.tile_pool(name="const", bufs=1) as const,
            tc.tile_pool(name="persist", bufs=1) as persist,
            tc.tile_pool(name="epool", bufs=3) as epool,
            tc.tile_pool(name="npool", bufs=2) as npool,
            tc.tile_pool(name="opool", bufs=4) as opool,
            # PSUM static budget (8 banks): pp 2 + s0 2 + s1 2 + c0 1 + c1 1
            tc.tile_pool(name="ppool", bufs=2, space="PSUM") as ppool,
            tc.tile_pool(name="spool", bufs=1, space="PSUM") as spool,
            tc.tile_pool(name="cpool", bufs=1, space="PSUM") as cpool,
        ):
            # ---- constants + resident x ----
            # first projection group needs wk chunks + x n-slice 0 only; those
            # DMAs go first, fine-grained, spread across the Sync / GpSimd /
            # Scalar queues so no single sequencer's DGE time serializes the
            # startup.
            wq_sb = const.tile([128, KC, DQ], F16, tag="wq")
            wk_sb = const.tile([128, KC, DQ], F16, tag="wk")
            wv_sb = const.tile([128, KC, DQ], F16, tag="wv")
            x_sb = const.tile([128, KC, N], F16, tag="x")
            xTr = xT.rearrange("(k p) n -> k p n", p=128)
            bk_sb = const.tile([128, 2], F32, tag="bk")
            bq_sb = const.tile([128, 2], F32, tag="bq")
            nc.scalar.dma_start(out=bk_sb, in_=bkd.rearrange("(m p) -> p m", p=128))
            nc.scalar.dma_start(out=bq_sb, in_=bqd.rearrange("(m p) -> p m", p=128))
            for k in range(KC):
                nc.sync.dma_start(out=wk_sb[:, k, :], in_=wkT.rearrange(
                    "(k p) d -> k p d", p=128)[k])
                nc.gpsimd.dma_start(out=x_sb[:, k, 0:NQ], in_=xTr[k, :, 0:NQ])
                nc.scalar.dma_start(out=wv_sb[:, k, :], in_=wvT.rearrange(
                    "(k p) d -> k p d", p=128)[k])
            for k in range(KC):
                nc.sync.dma_start(out=wq_sb[:, k, :], in_=wqT.rearrange(
                    "(k p) d -> k p d", p=128)[k])
                nc.gpsimd.dma_start(out=x_sb[:, k, 2 * NQ:3 * NQ],
                                    in_=xTr[k, :, 2 * NQ:3 * NQ])
                nc.scalar.dma_start(out=x_sb[:, k, NQ:2 * NQ],
                                    in_=xTr[k, :, NQ:2 * NQ])
            for k in range(KC):
                nc.sync.dma_start(out=x_sb[:, k, 3 * NQ:N],
                                  in_=xTr[k, :, 3 * NQ:N])
            # wo is needed only by the out-projection (~100us in) -- deferred
            # into the filler stream to keep startup queues clear
            wo_sb = const.tile([128, 2, EMB], F16, tag="wo")

            # ---- persistent activations ----
            qT = [persist.tile([128, N], F16, tag=f"qT{p}", name=f"qT{p}") for p in range(2)]
            kT = [persist.tile([128, N], F16, tag=f"kT{p}", name=f"kT{p}") for p in range(2)]
            ctxT = [persist.tile([128, N], F16, tag=f"ctxT{p}", name=f"ctxT{p}") for p in range(2)]
            # V for all 4 local heads: [nk-part, t, head*65 + (0:64 | ones)]
            v_all = persist.tile([128, NT, 4 * (HD + 1)], F16, tag="v")
            for h in range(4):
                nc.vector.memset(v_all[:, :, h * 65 + 64], 1.0)

            add, mult = mybir.AluOpType.add, mybir.AluOpType.mult

            # ---- projection building blocks ----
            # Each is one PSUM accumulation group on the double-buffered pp
            # tag, small enough to slot between attention groups.
            def kq_group(p, n, wsb, bsb, dst):
                ps = ppool.tile([128, NQ], F32, tag="pp", name="kqp")
                for k in range(KC):
                    nc.tensor.matmul(
                        ps, wsb[:, k, p * 128:(p + 1) * 128],
                        x_sb[:, k, n * NQ:(n + 1) * NQ],
                        start=(k == 0), stop=(k == KC - 1))
                nc.vector.tensor_tensor(
                    out=dst[p][:, n * NQ:(n + 1) * NQ], in0=ps,
                    in1=bsb[:, p:p + 1].broadcast_to([128, NQ]), op=add)

            def v_group(n, tl):
                # V for ALL 4 local heads at once (256-col moving wv)
                t = n * 4 + tl
                ps = ppool.tile([128, DQ], F32, tag="pp", name="vp")
                for k in range(KC):
                    nc.tensor.matmul(
                        ps, x_sb[:, k, t * 128:(t + 1) * 128],
                        wv_sb[:, k, :],
                        start=(k == 0), stop=(k == KC - 1))
                vv = v_all[:, t, :].rearrange("p (h c) -> p h c", c=65)
                nc.vector.tensor_copy(
                    out=vv[:, :, 0:64],
                    in_=ps.rearrange("p (h c) -> p h c", c=64))

            # Minimal cold prefix: only the work attention j=0 strictly
            # needs before its first items runs up front -- K(n=0), Q(n=0),
            # V(t=0,1). Everything else streams through the hot attention
            # windows as fillers.
            kq_group(0, 0, wk_sb, bk_sb, kT)
            kq_group(0, 0, wq_sb, bq_sb, qT)
            v_group(0, 0)
            v_group(0, 1)

            # ---- out-projection ----
            # one (m, eo) parcel: both head-pair passes accumulate into a
            # single pp-tag PSUM group, then one ACT copy to SBUF + store.
            def out_proj_parcel(m, eo):
                po = ppool.tile([128, NQ], F32, tag="pp", name="po")
                for kp in range(2):
                    nc.tensor.matmul(
                        po, ctxT[kp][:, m * 128:(m + 1) * 128],
                        wo_sb[:, kp, eo * NQ:(eo + 1) * NQ],
                        start=(kp == 0), stop=(kp == 1))
                o = opool.tile([128, NQ], F16, tag="o", name="o")
                nc.scalar.copy(out=o, in_=po)
                nc.sync.dma_start(
                    out=out_part[m * 128:(m + 1) * 128, eo * NQ:(eo + 1) * NQ],
                    in_=o)

            # The LAST window's m-chunks split the parcel: the ctxT0 half
            # runs during the window (ctxT0 is final since p0), leaving only
            # one matmul + add + store per parcel on the serial tail.
            o0s = {}

            def oproj_kp0(m, eo):
                po = ppool.tile([128, NQ], F32, tag="pp", name="po")
                nc.tensor.matmul(
                    po, ctxT[0][:, m * 128:(m + 1) * 128],
                    wo_sb[:, 0, eo * NQ:(eo + 1) * NQ], start=True, stop=True)
                o0 = opool.tile([128, NQ], F32, tag=f"o0_{m}_{eo}", name="o0")
                nc.scalar.copy(out=o0, in_=po)
                o0s[(m, eo)] = o0

            def oproj_kp1(m, eo):
                po = ppool.tile([128, NQ], F32, tag="pp", name="po")
                nc.tensor.matmul(
                    po, ctxT[1][:, m * 128:(m + 1) * 128],
                    wo_sb[:, 1, eo * NQ:(eo + 1) * NQ], start=True, stop=True)
                o = opool.tile([128, NQ], F16, tag="o", name="o")
                nc.vector.tensor_tensor(out=o, in0=o0s[(m, eo)], in1=po,
                                        op=add)
                nc.sync.dma_start(
                    out=out_part[m * 128:(m + 1) * 128, eo * NQ:(eo + 1) * NQ],
                    in_=o)

            # ---- attention (per head-pair p, nq window of 512/256) ----
            # Row-packed S + dual-engine exp; see module docstring.
            # Two filler queues: `fillers` (projection groups; no dependence
            # on fresh ctxT -> drain eagerly) and `parcels` (out-proj; read
            # ctxT columns written by the PREVIOUS window's normalize chain
            # -> drain only from item 4 so the PE never head-of-line blocks
            # on that chain).
            from collections import deque
            fillers = deque()
            parcels = deque()
            nfill = deque()  # deferred normalize steps of the previous window

            def K0(n):
                return lambda: kq_group(0, n, wk_sb, bk_sb, kT)

            def Q0(n):
                return lambda: kq_group(0, n, wq_sb, bq_sb, qT)

            def K1(n):
                return lambda: kq_group(1, n, wk_sb, bk_sb, kT)

            def Q1(n):
                return lambda: kq_group(1, n, wq_sb, bq_sb, qT)

            def V(n, tl):
                return lambda: v_group(n, tl)

            wo_dma = lambda: nc.sync.dma_start(  # noqa: E731
                out=wo_sb, in_=woT.rearrange("(k p) e -> p k e", p=128))

            # per-window static filler lists (window key = (p, index))
            sched = {
                (0, 0): ([V(0, 2), V(0, 3), K0(1)]
                         + [V(1, tl) for tl in range(4)] + [K0(2)]
                         + [V(2, tl) for tl in range(4)] + [K0(3)]
                         + [V(3, tl) for tl in range(2)] + [Q0(1)]
                         + [V(3, tl) for tl in range(2, 4)]),
                (0, 1): [Q0(2), K1(0), Q1(0)],
                (0, 2): [Q0(3), K1(1), wo_dma],
                (0, 3): [K1(2)],
                (1, 0): [K1(3), Q1(1)],
                (1, 1): [Q1(2)],
                (1, 2): [Q1(3)],
                (1, 3): [lambda m=m, eo=eo: oproj_kp0(m, eo)
                         for m in range(14, 16) for eo in range(2)],
            }

            for p in range(2):
                if p == 0:
                    wins = [(jq * NQA, NQA) for jq in range(NJA)]
                else:
                    # final window split in two 256-halves: the serial tail
                    # (normalize chain + kp1 finishers + stores) covers only
                    # 2 m-chunks
                    wins = [(jq * NQA, NQA) for jq in range(NJA - 1)]
                    wins += [(N - NQA, NQA // 2), (N - NQA // 2, NQA // 2)]
                for jw, (q0, w) in enumerate(wins):
                    statics = sched.get((p, jw), [])
                    fillers.extendleft(reversed(statics))
                    n_static = len(statics)
                    cps = [cpool.tile([HD + 1, w], F32, tag=f"c{h}",
                                      name=f"c{h}") for h in range(2)]

                    def s_mms_pair(g, q0=q0, w=w):
                        # both heads' S tiles, head-interleaved so adjacent
                        # matmuls hit disjoint row groups and run concurrent
                        sps = [spool.tile([128, GK, w], F32,
                                          tag=f"s{h}", name=f"s{h}")
                               for h in range(2)]
                        for i, t in enumerate(g):
                            for h in range(2):
                                lo = 64 * h
                                nc.tensor.matmul(
                                    sps[h][:, i, :],
                                    kT[p][lo:lo + 64, t * 128:(t + 1) * 128],
                                    qT[p][lo:lo + 64, q0:q0 + w],
                                    start=True, stop=True)
                        return sps

                    def exp_acts(sps, wi, w=w):
                        # one head's exp on ACT (true), the other on DVE
                        # (Schraudolph); roles alternate per item
                        ha = wi % 2
                        hd = 1 - ha
                        ea = epool.tile([128, GK, w], F16, tag="ea", name="ea")
                        nc.scalar.activation(ea, sps[ha], FP.Exp, scale=SCALE)
                        ed = epool.tile([128, GK, w], I16, tag="ed", name="ed")
                        nc.vector._custom_dve(EXP_OP, out=ed, in0=sps[hd],
                                              s0=A_EXP, s1=B_EXP)
                        es = [None, None]
                        es[ha] = ea
                        es[hd] = ed
                        return es

                    def ctx_mms(e, g, h):
                        hloc = 2 * p + h
                        for i, t in enumerate(g):
                            src = e[:, i, :]
                            if e.dtype == I16:
                                src = src.bitcast(F16)
                            nc.tensor.matmul(
                                cps[h],
                                v_all[:, t, hloc * 65:(hloc + 1) * 65],
                                src,
                                start=(t == 0), stop=(t == NT - 1))

                    work = [tuple(range(gi * GK, (gi + 1) * GK))
                            for gi in range(NG)]
                    n_pop = len(fillers)  # drain this window's statics fully
                    n_parcel = len(parcels)
                    popped = ppopped = 0
                    pend = deque()  # ctx lags TWO items behind S/exp
                    for wi, g in enumerate(work):
                        # order within an item: ctx(u-2) FIRST (its e tiles
                        # are certainly ready, so the PE always has ready
                        # work in front of S(u)'s s-tag wait), then S(u),
                        # exps(u), deferred-normalize step, fillers.
                        if len(pend) == 2:
                            es, gp = pend.popleft()
                            for h in range(2):
                                ctx_mms(es[h], gp, h)
                        sps = s_mms_pair(g)
                        pend.append((exp_acts(sps, wi), g))
                        if wi >= 1 and nfill:
                            nfill.popleft()()
                        target = max(min(2 * (wi + 1), n_static),
                                     (wi + 1) * n_pop // max(1, len(work) - 1))
                        while fillers and popped < min(n_pop, target):
                            fillers.popleft()()
                            popped += 1
                        if wi >= 5:
                            ptarget = (wi - 4) * n_parcel // (len(work) - 5)
                            while parcels and ppopped < min(n_parcel, ptarget):
                                parcels.popleft()()
                                ppopped += 1
                    while pend:
                        es, gp = pend.popleft()
                        for h in range(2):
                            ctx_mms(es[h], gp, h)
                    while fillers and popped < n_pop:
                        fillers.popleft()()
                        popped += 1
                    while parcels and ppopped < n_parcel:
                        parcels.popleft()()
                        ppopped += 1

                    # normalize: ctx^T[0:64] * (1 / rowsum); rowsum in row 64.
                    # rowsum copy + ctx staging run now (release the c PSUM
                    # banks); the chain broadcast -> reciprocal -> multiply
                    # is DEFERRED into items 1-3 of the NEXT window so the
                    # gpsimd wait never head-of-line-blocks the DVE exp
                    # stream at the window boundary. Last window runs the
                    # chain inline (straight from PSUM; gates only the tail).
                    last = (p == 1 and jw == len(wins) - 1)
                    rss = [None, None]
                    css = [None, None]
                    for h in range(2):
                        rs = npool.tile([1, w], F32, tag="rs", name="rs")
                        nc.vector.tensor_copy(rs, cps[h][64:65, :])
                        rss[h] = rs
                        if not last:
                            # stage ctx to SBUF (ACT: the DVE queue is the
                            # exp critical path) to release the c PSUM bank
                            # for the next window
                            cs = npool.tile([64, w], F32, tag="cs", name="cs")
                            nc.scalar.copy(out=cs, in_=cps[h][0:64, :])
                            css[h] = cs

                    def n_bcast(rss=rss, w=w):
                        rbs = []
                        for h in range(2):
                            rb = npool.tile([64, w], F32, tag=f"rb{h}",
                                            name="rb")
                            nc.gpsimd.partition_broadcast(rb, rss[h])
                            rbs.append(rb)
                        return rbs

                    def n_recip(rbs, w=w):
                        rcs = []
                        for h in range(2):
                            rc = npool.tile([64, w], F32, tag=f"rc{h}",
                                            name="rc")
                            nc.vector.reciprocal_approx_fast(out=rc, in_=rbs[h])
                            rcs.append(rc)
                        return rcs

                    def n_mult(rcs, css=css, p=p, q0=q0, w=w):
                        for h in range(2):
                            nc.vector.tensor_tensor(
                                out=ctxT[p][h * 64:(h + 1) * 64, q0:q0 + w],
                                in0=css[h], in1=rcs[h], op=mult)

                    if last:
                        rbs = n_bcast()
                        rcs = n_recip(rbs)
                        for h in range(2):
                            nc.vector.tensor_tensor(
                                out=ctxT[p][h * 64:(h + 1) * 64, q0:q0 + w],
                                in0=cps[h][0:64, :], in1=rcs[h], op=mult)
                    else:
                        box = {}
                        nfill.append(lambda box=box: box.__setitem__(
                            "rbs", n_bcast()))
                        nfill.append(lambda box=box: box.__setitem__(
                            "rcs", n_recip(box["rbs"])))
                        nfill.append(lambda box=box: n_mult(box["rcs"]))
                    if p == 1:
                        # ctxT1 columns for this window are final -> out-proj
                        # parcels for the covered m-chunks can run (delayed
                        # to item>=4 of the next window by the parcel queue)
                        for m in range(q0 // 128, (q0 + w) // 128):
                            for eo in range(2):
                                f = out_proj_parcel if jw < len(wins) - 1 else oproj_kp1
                                parcels.append(lambda m=m, eo=eo, f=f: f(m, eo))
            while nfill or fillers or parcels:
                q = nfill if nfill else (fillers if fillers else parcels)
                q.popleft()()

    nc.compile()
    return nc


_NC_CACHE = {}


def _get_program():
    if "nc" not in _NC_CACHE:
        _NC_CACHE["nc"] = build_program()
    return _NC_CACHE["nc"]


def make_in_maps(x, Wq, bq, Wk, bk, Wv, bv, Wo, bo):
    x = np.asarray(x)
    xTs = [np.ascontiguousarray(x[b].T.astype(np.float16)) for b in range(B)]
    in_maps = []
    for c in range(N_CORES):
        b, hg = divmod(c, TP)
        sl = slice(hg * DQ, (hg + 1) * DQ)
        in_maps.append({
            "xT": xTs[b],
            "wqT": np.ascontiguousarray(np.asarray(Wq, np.float16)[sl, :].T),
            "wkT": np.ascontiguousarray(np.asarray(Wk, np.float16)[sl, :].T),
            "wvT": np.ascontiguousarray(np.asarray(Wv, np.float16)[sl, :].T),
            "woT": np.ascontiguousarray(np.asarray(Wo, np.float16)[:, sl].T),
            "bq_s": np.ascontiguousarray(np.asarray(bq, np.float32)[sl]),
            "bk_s": np.ascontiguousarray(np.asarray(bk, np.float32)[sl]),
        })
    return in_maps


def assemble_output(results, Wv_bias_term):
    out = np.empty((B, N, EMB), np.float32)
    for b in range(B):
        acc = results[b * TP]["out_part"].astype(np.float32)
        for g in range(1, TP):
            acc = acc + results[b * TP + g]["out_part"]
        out[b] = acc + Wv_bias_term
    return out


def kernel(x, Wq, bq, Wk, bk, Wv, bv, Wo, bo):
    nc = _get_program()
    in_maps = make_in_maps(x, Wq, bq, Wk, bk, Wv, bv, Wo, bo)
    res = run_bass_kernel_spmd(nc, in_maps, list(range(N_CORES)))
    bias_term = (np.asarray(bv, np.float32) @ np.asarray(Wo, np.float32).T
                 + np.asarray(bo, np.float32))
    return assemble_output(res.results, bias_term)


# revision 26
# speedup vs baseline: 1.0451x; 1.0326x over previous
"""Multi-head self-attention (b=2, n=2048, emb=1024, heads=16) on 8 trn2 cores.

Sharding: core c = (b, hg) with b = c // 4, hg = c % 4. Data parallel over
batch, tensor parallel over head-groups (4 heads / 256 emb-cols per core).
Each core computes Q/K/V projections for its heads, full attention for its
heads, and a partial output projection ctx_hg @ Wo[:, hg_slice].T of shape
[2048, 1024]. The host sums the 4 partials per batch (Megatron row-parallel
reduce done on host) and adds the rank-1 bias term bv @ Wo.T + bo.

Key device-side structure (v2):
- x^T resident in SBUF; fp16 matmuls everywhere; q/k biases fused into the
  PSUM->SBUF copies; v/o biases are the host-side rank-1 term.
- S matmuls are ROW-PACKED: the two heads of a pair live in partition rows
  0:64 / 64:128 of qT/kT, and their S matmuls are emitted head-interleaved
  so adjacent matmuls target disjoint 64-row groups of the PE array
  (tile_position auto-derived from base_partition) and stream concurrently
  -> a packed pair costs ~512 cycles instead of 2x512.
- The two heads' exps run on DIFFERENT engines concurrently: one on ACT
  (true exp, 1024-wide out of PSUM) and one on DVE via a custom
  Schraudolph op: e = bitcast_fp16(int16(relu(S*(scale*1024/ln2)+15315.3)))
  (max rel err ~3%, mean ~1.8%, HW-validated). Which head uses which
  engine alternates per work item, so the approx noise averages across
  nk-chunks inside every head instead of concentrating in one head
  (absmax-rel error ~2.4e-3 in numpy sim). This halves the exp-chain
  period AND lets both S psum tags recycle each item.
- ctx matmuls are emitted FIRST in each item, lagging TWO items behind
  their S/exp, so the in-order PE queue always holds ready work in front
  of S's psum-tag wait.
- V carries a ones column per head -> ctx matmul row 64 accumulates the
  softmax denominators for free. The normalize chain (rowsum -> gpsimd
  partition_broadcast -> reciprocal_approx_fast -> multiply, DVE ops) is
  DEFERRED into items 1-3 of the next window so its gpsimd wait never
  head-of-line-blocks the DVE exp stream at a window boundary; the ctx
  PSUM banks are released immediately via an ACT staging copy. The last
  window runs the chain inline straight from PSUM (shortest tail).
- Engine balance (learned the hard way: the DVE queue IS the exp critical
  path, keep auxiliary copies off it): ctx staging + out-proj PSUM->SBUF
  copies on ACT; kq-bias adds, v copies, reciprocal, normalize multiplies
  on DVE; out_part stores issue on the sync DMA queue.
- Projection/out-proj parcels fill PE slack inside the attention windows
  from per-window filler lists (emission-deadline ordered); out-proj
  parcels drain only from item 4 so they never wait on the deferred
  normalize of the previous window.
"""

import os
import sys

for _p in ("/opt/trn_rl_repo", "/root/.axon_site/_ro/trn_rl_repo"):
    if os.path.isdir(_p) and _p not in sys.path:
        sys.path.append(_p)

import numpy as np

import concourse.bass as bass  # noqa: F401  (engine types pulled via nc)
import concourse.mybir as mybir
import concourse.tile as tile
from concourse import bacc
from concourse.bass_utils import run_bass_kernel_spmd
from concourse import dve_ops
from concourse.dve_spec import Spec, Src0, C0, C1, relu, lower
from concourse.dve_uop import DveOpSpec

B, N, EMB, HEADS, HD = 2, 2048, 1024, 16, 64
N_CORES = 8
TP = 4                      # head-group shards per batch
DQ = EMB // TP              # 256 emb-cols (4 heads) per core
SCALE = HD ** -0.5          # 0.125

F32 = mybir.dt.float32
F16 = mybir.dt.float16
I16 = mybir.dt.int16
FP = mybir.ActivationFunctionType

NQ = 512                    # nq chunk for projections / out-proj
NJ = N // NQ                # 4 nq chunks
NQA = 512                   # nq chunk for attention
NJA = N // NQA              # 4 attention nq chunks
NKC = 128                   # nk chunk (ctx contraction)
NT = N // NKC               # 16 nk chunks
KC = EMB // 128             # 8 e chunks
GK = 2                      # nk chunks per S-psum group (1024-wide exps)
NG = NT // GK               # 8 groups per (head-pair, j)

# Schraudolph fp16-bit exp: bits = relu(x*SCALE*1024/ln2 + (15360-44.7))
A_EXP = float(SCALE * 1024.0 / np.log(2.0))
B_EXP = 15360.0 - 44.7


def _register_exp_op():
    for op in dve_ops.OPS:
        if op.name == "EXP_SCHRAUDOLPH_ANT":
            return op

    def _ref(in0, in1, c0, c1, c2):
        return np.maximum(in0 * c0 + c1, 0.0)

    spec = Spec(body=relu(Src0 * C0 + C1), reference=_ref)
    shas = {}
    for ver in ("v3", "v4"):
        tmp = DveOpSpec(name="EXP_SCHRAUDOLPH_ANT", opcode=1,
                        uops=lower(spec, ver=ver), rd1_en=False)
        shas[ver] = tmp.sha(ver)
    op = dve_ops.DveOp("EXP_SCHRAUDOLPH_ANT", spec, subdim=False, uops_sha=shas)
    dve_ops.OPS.append(op)
    dve_ops.CUSTOM_DVE_SPECS[op.name] = op.spec
    dve_ops._SUB_OPCODE_FOR_NAME[op.name] = (
        dve_ops._CUSTOM_DVE_ROW_BASE + len(dve_ops.OPS) - 1)
    return op


EXP_OP = _register_exp_op()


def build_program():
    """Build + compile the single SPMD program all 8 cores run."""
    nc = bacc.Bacc("TRN2", target_bir_lowering=False, debug=False,
                   num_devices=N_CORES)

    xT = nc.dram_tensor("xT", [EMB, N], F16, kind="ExternalInput").ap()
    wqT = nc.dram_tensor("wqT", [EMB, DQ], F16, kind="ExternalInput").ap()
    wkT = nc.dram_tensor("wkT", [EMB, DQ], F16, kind="ExternalInput").ap()
    wvT = nc.dram_tensor("wvT", [EMB, DQ], F16, kind="ExternalInput").ap()
    woT = nc.dram_tensor("woT", [DQ, EMB], F16, kind="ExternalInput").ap()
    bqd = nc.dram_tensor("bq_s", [DQ], F32, kind="ExternalInput").ap()
    bkd = nc.dram_tensor("bk_s", [DQ], F32, kind="ExternalInput").ap()
    # fp16 partials: host sums 4 of them in fp32; quantization of the
    # partial (|.| ~ 1, ulp ~ 1e-3) adds ~1e-3 absmax-relative error --
    # well under the 2e-2 gate -- and halves the store traffic + tail.
    out_part = nc.dram_tensor("out_part", [N, EMB], F16,
                              kind="ExternalOutput").ap()

    with tile.TileContext(nc) as tc:
        with (
            tc.tile_pool(name="const", bufs=1) as const,
            tc.tile_pool(name="persist", bufs=1) as persist,
            tc.tile_pool(name="epool", bufs=4) as epool,
            tc.tile_pool(name="npool", bufs=2) as npool,
            tc.tile_pool(name="opool", bufs=4) as opool,
            # PSUM static budget (8 banks): pp 2 + s0 2 + s1 2 + c0 1 + c1 1
            tc.tile_pool(name="ppool", bufs=2, space="PSUM") as ppool,
            tc.tile_pool(name="spool", bufs=1, space="PSUM") as spool,
            tc.tile_pool(name="cpool", bufs=1, space="PSUM") as cpool,
        ):
            # ---- constants + resident x ----
            # first projection group needs wk chunks + x n-slice 0 only; those
            # DMAs go first, fine-grained, spread across the Sync / GpSimd /
            # Scalar queues so no single sequencer's DGE time serializes the
            # startup.
            wq_sb = const.tile([128, KC, DQ], F16, tag="wq")
            wk_sb = const.tile([128, KC, DQ], F16, tag="wk")
            wv_sb = const.tile([128, KC, DQ], F16, tag="wv")
            x_sb = const.tile([128, KC, N], F16, tag="x")
            xTr = xT.rearrange("(k p) n -> k p n", p=128)
            bk_sb = const.tile([128, 2], F32, tag="bk")
            bq_sb = const.tile([128, 2], F32, tag="bq")
            nc.scalar.dma_start(out=bk_sb, in_=bkd.rearrange("(m p) -> p m", p=128))
            nc.scalar.dma_start(out=bq_sb, in_=bqd.rearrange("(m p) -> p m", p=128))
            # wave 1 carries the interleaved first K/Q group's needs
            # (wk + wq + x0) in per-chunk arrival order; wv and the later
            # x slices follow before their first consumers.
            wkr = wkT.rearrange("(k p) d -> k p d", p=128)
            wqr = wqT.rearrange("(k p) d -> k p d", p=128)
            wvr = wvT.rearrange("(k p) d -> k p d", p=128)
            for k in range(KC):
                nc.sync.dma_start(out=wk_sb[:, k, :], in_=wkr[k])
                nc.gpsimd.dma_start(out=x_sb[:, k, 0:NQ], in_=xTr[k, :, 0:NQ])
                nc.scalar.dma_start(out=wq_sb[:, k, :], in_=wqr[k])
            for k in range(KC):
                nc.sync.dma_start(out=x_sb[:, k, NQ:2 * NQ],
                                  in_=xTr[k, :, NQ:2 * NQ])
                nc.gpsimd.dma_start(out=wv_sb[:, k, :], in_=wvr[k])
                nc.scalar.dma_start(out=x_sb[:, k, 2 * NQ:3 * NQ],
                                    in_=xTr[k, :, 2 * NQ:3 * NQ])
            for k in range(KC):
                nc.gpsimd.dma_start(out=x_sb[:, k, 3 * NQ:N],
                                    in_=xTr[k, :, 3 * NQ:N])
            # wo is needed only by the out-projection (~100us in) -- deferred
            # into the filler stream to keep startup queues clear
            wo_sb = const.tile([128, 2, EMB], F16, tag="wo")

            # ---- persistent activations ----
            qT = [persist.tile([128, N], F16, tag=f"qT{p}", name=f"qT{p}") for p in range(2)]
            kT = [persist.tile([128, N], F16, tag=f"kT{p}", name=f"kT{p}") for p in range(2)]
            ctxT = [persist.tile([128, N], F16, tag=f"ctxT{p}", name=f"ctxT{p}") for p in range(2)]
            # V for all 4 local heads: [nk-part, t, head*65 + (0:64 | ones)]
            v_all = persist.tile([128, NT, 4 * (HD + 1)], F16, tag="v")
            for h in range(4):
                nc.vector.memset(v_all[:, :, h * 65 + 64], 1.0)

            add, mult = mybir.AluOpType.add, mybir.AluOpType.mult

            # ---- projection building blocks ----
            # Each is one PSUM accumulation group on the double-buffered pp
            # tag, small enough to slot between attention groups.
            def kq_group(p, n, wsb, bsb, dst):
                ps = ppool.tile([128, NQ], F32, tag="pp", name="kqp")
                for k in range(KC):
                    nc.tensor.matmul(
                        ps, wsb[:, k, p * 128:(p + 1) * 128],
                        x_sb[:, k, n * NQ:(n + 1) * NQ],
                        start=(k == 0), stop=(k == KC - 1))
                nc.vector.tensor_tensor(
                    out=dst[p][:, n * NQ:(n + 1) * NQ], in0=ps,
                    in1=bsb[:, p:p + 1].broadcast_to([128, NQ]), op=add)

            def v_group(n, tl):
                # V for ALL 4 local heads at once (256-col moving wv)
                t = n * 4 + tl
                ps = ppool.tile([128, DQ], F32, tag="pp", name="vp")
                for k in range(KC):
                    nc.tensor.matmul(
                        ps, x_sb[:, k, t * 128:(t + 1) * 128],
                        wv_sb[:, k, :],
                        start=(k == 0), stop=(k == KC - 1))
                vv = v_all[:, t, :].rearrange("p (h c) -> p h c", c=65)
                nc.vector.tensor_copy(
                    out=vv[:, :, 0:64],
                    in_=ps.rearrange("p (h c) -> p h c", c=64))

            # Minimal cold prefix: only the work attention j=0 strictly
            # needs before its first items runs up front -- K(n=0), Q(n=0),
            # V(t=0,1). K and Q interleave per k-chunk (both pp banks) so
            # they consume x0/wk/wq chunks as the startup DMAs land instead
            # of running serially.
            psK = ppool.tile([128, NQ], F32, tag="pp", name="kqp")
            psQ = ppool.tile([128, NQ], F32, tag="pp", name="kqp")
            for k in range(KC):
                nc.tensor.matmul(psK, wk_sb[:, k, 0:128], x_sb[:, k, 0:NQ],
                                 start=(k == 0), stop=(k == KC - 1))
                nc.tensor.matmul(psQ, wq_sb[:, k, 0:128], x_sb[:, k, 0:NQ],
                                 start=(k == 0), stop=(k == KC - 1))
            nc.vector.tensor_tensor(
                out=kT[0][:, 0:NQ], in0=psK,
                in1=bk_sb[:, 0:1].broadcast_to([128, NQ]), op=add)
            nc.vector.tensor_tensor(
                out=qT[0][:, 0:NQ], in0=psQ,
                in1=bq_sb[:, 0:1].broadcast_to([128, NQ]), op=add)
            v_group(0, 0)
            v_group(0, 1)

            # ---- out-projection ----
            # one (m, eo) parcel: both head-pair passes accumulate into a
            # single pp-tag PSUM group, then one ACT copy to SBUF + store.
            def out_proj_parcel(m, eo):
                po = ppool.tile([128, NQ], F32, tag="pp", name="po")
                for kp in range(2):
                    nc.tensor.matmul(
                        po, ctxT[kp][:, m * 128:(m + 1) * 128],
                        wo_sb[:, kp, eo * NQ:(eo + 1) * NQ],
                        start=(kp == 0), stop=(kp == 1))
                o = opool.tile([128, NQ], F16, tag="o", name="o")
                nc.scalar.copy(out=o, in_=po)
                nc.sync.dma_start(
                    out=out_part[m * 128:(m + 1) * 128, eo * NQ:(eo + 1) * NQ],
                    in_=o)

            # The LAST window's m-chunks split the parcel: the ctxT0 half
            # runs during the window (ctxT0 is final since p0), leaving only
            # one matmul + add + store per parcel on the serial tail.
            o0s = {}

            def oproj_kp0(m, eo):
                po = ppool.tile([128, NQ], F32, tag="pp", name="po")
                nc.tensor.matmul(
                    po, ctxT[0][:, m * 128:(m + 1) * 128],
                    wo_sb[:, 0, eo * NQ:(eo + 1) * NQ], start=True, stop=True)
                o0 = opool.tile([128, NQ], F32, tag=f"o0_{m}_{eo}", name="o0")
                nc.scalar.copy(out=o0, in_=po)
                o0s[(m, eo)] = o0

            def oproj_kp1(m, eo):
                po = ppool.tile([128, NQ], F32, tag="pp", name="po")
                nc.tensor.matmul(
                    po, ctxT[1][:, m * 128:(m + 1) * 128],
                    wo_sb[:, 1, eo * NQ:(eo + 1) * NQ], start=True, stop=True)
                o = opool.tile([128, NQ], F16, tag="o", name="o")
                nc.vector.tensor_tensor(out=o, in0=o0s[(m, eo)], in1=po,
                                        op=add)
                nc.sync.dma_start(
                    out=out_part[m * 128:(m + 1) * 128, eo * NQ:(eo + 1) * NQ],
                    in_=o)

            # ---- attention (per head-pair p, nq window of 512/256) ----
            # Row-packed S + dual-engine exp; see module docstring.
            # Two filler queues: `fillers` (projection groups; no dependence
            # on fresh ctxT -> drain eagerly) and `parcels` (out-proj; read
            # ctxT columns written by the PREVIOUS window's normalize chain
            # -> drain only from item 4 so the PE never head-of-line blocks
            # on that chain).
            from collections import deque
            fillers = deque()
            parcels = deque()
            nfill = deque()  # deferred normalize steps of the previous window

            def K0(n):
                return lambda: kq_group(0, n, wk_sb, bk_sb, kT)

            def Q0(n):
                return lambda: kq_group(0, n, wq_sb, bq_sb, qT)

            def K1(n):
                return lambda: kq_group(1, n, wk_sb, bk_sb, kT)

            def Q1(n):
                return lambda: kq_group(1, n, wq_sb, bq_sb, qT)

            def V(n, tl):
                return lambda: v_group(n, tl)

            wo_dma = lambda: nc.sync.dma_start(  # noqa: E731
                out=wo_sb, in_=woT.rearrange("(k p) e -> p k e", p=128))

            # per-window static filler lists (window key = (p, index))
            sched = {
                (0, 0): ([V(0, 2), V(0, 3), K0(1)]
                         + [V(1, tl) for tl in range(4)] + [K0(2)]
                         + [V(2, tl) for tl in range(4)] + [K0(3)]
                         + [V(3, tl) for tl in range(2)] + [Q0(1)]
                         + [V(3, tl) for tl in range(2, 4)]),
                (0, 1): [Q0(2), K1(0), Q1(0)],
                (0, 2): [Q0(3), K1(1), wo_dma],
                (0, 3): [K1(2)],
                (1, 0): [K1(3), Q1(1)],
                (1, 1): [Q1(2)],
                (1, 2): [Q1(3)],
                (1, 3): [lambda m=m, eo=eo: oproj_kp0(m, eo)
                         for m in range(14, 16) for eo in range(2)],
            }

            for p in range(2):
                if p == 0:
                    wins = [(jq * NQA, NQA) for jq in range(NJA)]
                else:
                    # final window split in two 256-halves: the serial tail
                    # (normalize chain + kp1 finishers + stores) covers only
                    # 2 m-chunks
                    wins = [(jq * NQA, NQA) for jq in range(NJA - 1)]
                    wins += [(N - NQA, NQA // 2), (N - NQA // 2, NQA // 2)]
                for jw, (q0, w) in enumerate(wins):
                    statics = sched.get((p, jw), [])
                    fillers.extendleft(reversed(statics))
                    n_static = len(statics)
                    cps = [cpool.tile([HD + 1, w], F32, tag=f"c{h}",
                                      name=f"c{h}") for h in range(2)]

                    def s_mms_pair(g, q0=q0, w=w):
                        # both heads' S tiles, head-interleaved so adjacent
                        # matmuls hit disjoint row groups and run concurrent
                        sps = [spool.tile([128, GK, w], F32,
                                          tag=f"s{h}", name=f"s{h}")
                               for h in range(2)]
                        for i, t in enumerate(g):
                            for h in range(2):
                                lo = 64 * h
                                nc.tensor.matmul(
                                    sps[h][:, i, :],
                                    kT[p][lo:lo + 64, t * 128:(t + 1) * 128],
                                    qT[p][lo:lo + 64, q0:q0 + w],
                                    start=True, stop=True)
                        return sps

                    def exp_acts(sps, wi, w=w):
                        # one head's exp on ACT (true), the other on DVE
                        # (Schraudolph); roles alternate per item
                        ha = wi % 2
                        hd = 1 - ha
                        ea = epool.tile([128, GK, w], F16, tag="ea", name="ea")
                        nc.scalar.activation(ea, sps[ha], FP.Exp, scale=SCALE)
                        ed = epool.tile([128, GK, w], I16, tag="ed", name="ed")
                        nc.vector._custom_dve(EXP_OP, out=ed, in0=sps[hd],
                                              s0=A_EXP, s1=B_EXP)
                        es = [None, None]
                        es[ha] = ea
                        es[hd] = ed
                        return es

                    def ctx_mms(e, g, h):
                        hloc = 2 * p + h
                        for i, t in enumerate(g):
                            src = e[:, i, :]
                            if e.dtype == I16:
                                src = src.bitcast(F16)
                            nc.tensor.matmul(
                                cps[h],
                                v_all[:, t, hloc * 65:(hloc + 1) * 65],
                                src,
                                start=(t == 0), stop=(t == NT - 1))

                    work = [tuple(range(gi * GK, (gi + 1) * GK))
                            for gi in range(NG)]
                    n_pop = len(fillers)  # drain this window's statics fully
                    n_parcel = len(parcels)
                    popped = ppopped = 0
                    pend = deque()  # ctx lags THREE items behind S/exp
                    for wi, g in enumerate(work):
                        # order within an item: ctx(u-3) FIRST (its e tiles
                        # are certainly ready, so the PE always has ready
                        # work in front of S(u)'s s-tag wait), then S(u),
                        # exps(u), deferred-normalize step, fillers.
                        if len(pend) == 3:
                            es, gp = pend.popleft()
                            for h in range(2):
                                ctx_mms(es[h], gp, h)
                        sps = s_mms_pair(g)
                        pend.append((exp_acts(sps, wi), g))
                        if wi >= 1 and nfill:
                            nfill.popleft()()
                        target = max(min(2 * (wi + 1), n_static),
                                     (wi + 1) * n_pop // max(1, len(work) - 1))
                        while fillers and popped < min(n_pop, target):
                            fillers.popleft()()
                            popped += 1
                        if wi >= 5:
                            ptarget = (wi - 4) * n_parcel // (len(work) - 5)
                            while parcels and ppopped < min(n_parcel, ptarget):
                                parcels.popleft()()
                                ppopped += 1
                    while pend:
                        es, gp = pend.popleft()
                        for h in range(2):
                            ctx_mms(es[h], gp, h)
                    while fillers and popped < n_pop:
                        fillers.popleft()()
                        popped += 1
                    while parcels and ppopped < n_parcel:
                        parcels.popleft()()
                        ppopped += 1

                    # normalize: ctx^T[0:64] * (1 / rowsum); rowsum in row 64.
                    # rowsum copy + ctx staging run now (release the c PSUM
                    # banks); the chain broadcast -> reciprocal -> multiply
                    # is DEFERRED into items 1-3 of the NEXT window so the
                    # gpsimd wait never head-of-line-blocks the DVE exp
                    # stream at the window boundary. Last window runs the
                    # chain inline (straight from PSUM; gates only the tail).
                    last = (p == 1 and jw == len(wins) - 1)
                    rss = [None, None]
                    css = [None, None]
                    for h in range(2):
                        rs = npool.tile([1, w], F32, tag="rs", name="rs")
                        nc.vector.tensor_copy(rs, cps[h][64:65, :])
                        rss[h] = rs
                        if not last:
                            # stage ctx to SBUF (ACT: the DVE queue is the
                            # exp critical path) to release the c PSUM bank
                            # for the next window
                            cs = npool.tile([64, w], F32, tag="cs", name="cs")
                            nc.scalar.copy(out=cs, in_=cps[h][0:64, :])
                            css[h] = cs

                    def n_bcast(rss=rss, w=w):
                        rbs = []
                        for h in range(2):
                            rb = npool.tile([64, w], F32, tag=f"rb{h}",
                                            name="rb")
                            nc.gpsimd.partition_broadcast(rb, rss[h])
                            rbs.append(rb)
                        return rbs

                    def n_recip(rbs, w=w):
                        rcs = []
                        for h in range(2):
                            rc = npool.tile([64, w], F32, tag=f"rc{h}",
                                            name="rc")
                            nc.vector.reciprocal_approx_fast(out=rc, in_=rbs[h])
                            rcs.append(rc)
                        return rcs

                    def n_mult(rcs, css=css, p=p, q0=q0, w=w):
                        for h in range(2):
                            nc.vector.tensor_tensor(
                                out=ctxT[p][h * 64:(h + 1) * 64, q0:q0 + w],
                                in0=css[h], in1=rcs[h], op=mult)

                    if last:
                        rbs = n_bcast()
                        rcs = n_recip(rbs)
                        for h in range(2):
                            nc.vector.tensor_tensor(
                                out=ctxT[p][h * 64:(h + 1) * 64, q0:q0 + w],
                                in0=cps[h][0:64, :], in1=rcs[h], op=mult)
                    else:
                        box = {}
                        nfill.append(lambda box=box: box.__setitem__(
                            "rbs", n_bcast()))
                        nfill.append(lambda box=box: box.__setitem__(
                            "rcs", n_recip(box["rbs"])))
                        nfill.append(lambda box=box: n_mult(box["rcs"]))
                    if p == 1:
                        # ctxT1 columns for this window are final -> out-proj
                        # parcels for the covered m-chunks can run (delayed
                        # to item>=4 of the next window by the parcel queue)
                        for m in range(q0 // 128, (q0 + w) // 128):
                            for eo in range(2):
                                f = out_proj_parcel if jw < len(wins) - 1 else oproj_kp1
                                parcels.append(lambda m=m, eo=eo, f=f: f(m, eo))
            while nfill or fillers or parcels:
                q = nfill if nfill else (fillers if fillers else parcels)
                q.popleft()()

    nc.compile()
    return nc


_NC_CACHE = {}


def _get_program():
    if "nc" not in _NC_CACHE:
        _NC_CACHE["nc"] = build_program()
    return _NC_CACHE["nc"]


def make_in_maps(x, Wq, bq, Wk, bk, Wv, bv, Wo, bo):
    x = np.asarray(x)
    xTs = [np.ascontiguousarray(x[b].T.astype(np.float16)) for b in range(B)]
    in_maps = []
    for c in range(N_CORES):
        b, hg = divmod(c, TP)
        sl = slice(hg * DQ, (hg + 1) * DQ)
        in_maps.append({
            "xT": xTs[b],
            "wqT": np.ascontiguousarray(np.asarray(Wq, np.float16)[sl, :].T),
            "wkT": np.ascontiguousarray(np.asarray(Wk, np.float16)[sl, :].T),
            "wvT": np.ascontiguousarray(np.asarray(Wv, np.float16)[sl, :].T),
            "woT": np.ascontiguousarray(np.asarray(Wo, np.float16)[:, sl].T),
            "bq_s": np.ascontiguousarray(np.asarray(bq, np.float32)[sl]),
            "bk_s": np.ascontiguousarray(np.asarray(bk, np.float32)[sl]),
        })
    return in_maps


def assemble_output(results, Wv_bias_term):
    out = np.empty((B, N, EMB), np.float32)
    for b in range(B):
        acc = results[b * TP]["out_part"].astype(np.float32)
        for g in range(1, TP):
            acc = acc + results[b * TP + g]["out_part"]
        out[b] = acc + Wv_bias_term
    return out


def kernel(x, Wq, bq, Wk, bk, Wv, bv, Wo, bo):
    nc = _get_program()
    in_maps = make_in_maps(x, Wq, bq, Wk, bk, Wv, bv, Wo, bo)
    res = run_bass_kernel_spmd(nc, in_maps, list(range(N_CORES)))
    bias_term = (np.asarray(bv, np.float32) @ np.asarray(Wo, np.float32).T
                 + np.asarray(bo, np.float32))
    return assemble_output(res.results, bias_term)
